# revision 38
# baseline (speedup 1.0000x reference)
"""Trainium2 Bass kernel for nn_ChunkwiseMLSTM (B=2, S=8192, D=512, INNER=1024, NH=8).

kernel(**inputs) -> np.ndarray [2, 8192, 512] f32.

Three SPMD launches on 8 NeuronCores:
  A: token-sharded projections (up-proj, causal conv+SiLU split across
     DVE/Pool with 4x tensor_scalar taps, q/k/v, gate pre-acts, skip*xm,
     silu(x_og) fused into single ACT ops straight from PSUM).
  B: head-sharded chunkwise mLSTM (LC=128 chunks):
     - stage i: inter-chunk state updates U_c = kesc_c^T @ [v|1] (PE),
       strided PSUM->SBUF evacs (ACT/Pool), then per-unit e-split
       tensor_tensor_scan on DVE+Pool with broadcast-AP decay input.
     - stage ii: per 4-chunk blocks: S matmuls (PE), mask-mult, num
       matmuls, f32 den column extraction; den clamp math batched per
       16-chunk group; h scaling + group-batched DMA out (Pool queue).
  C: token-sharded output gating + down-projection, 512-token-sliced
     streaming; host pre-folds h+skip*xm during the B->C reshard.
Host between launches: gate-derived scan scalars (O(B*NH*S)), weight
pre-transposition, resharding.
"""
import os
os.environ.setdefault("JAX_COMPILATION_CACHE_DIR",
                      os.path.expanduser("~/.cache/jax_bass_cache"))
os.environ.setdefault("JAX_PERSISTENT_CACHE_MIN_ENTRY_SIZE_BYTES", "0")
os.environ.setdefault("JAX_PERSISTENT_CACHE_MIN_COMPILE_TIME_SECS", "0")

import sys
if '/opt/trn_rl_repo' not in sys.path:
    sys.path.insert(0, '/opt/trn_rl_repo')

import numpy as np
import ml_dtypes

import concourse.bass as bass
import concourse.tile as tile
from concourse import mybir, bacc

F32 = mybir.dt.float32
BF16 = mybir.dt.bfloat16
AF = mybir.ActivationFunctionType
OP = mybir.AluOpType

B, S, D = 2, 8192, 512
INNER, NH, KCONV = 1024, 8, 4
DH = 128
EPS = 1e-6
LC = 128           # chunk length used on device (math is chunk-size invariant)
NCH = S // LC      # 64
NW = 129           # [C | n] state width
QK_SCALE = DH ** -0.5
TOK = S // 4       # tokens per core in phases A/C = 2048
TH = TOK + (KCONV - 1)   # 2051 with conv halo
NUNIT = 2          # (b,h) units per core in phase B


def _bf(x):
    return np.ascontiguousarray(np.asarray(x, np.float32).astype(ml_dtypes.bfloat16))


def new_nc():
    return bacc.Bacc(None, target_bir_lowering=False, debug=False)


# ---------------------------------------------------------------- phase A ----
def build_phase_a():
    nc = new_nc()
    xt = nc.dram_tensor("xt", [D, TH], BF16, kind="ExternalInput")            # x[b].T slice (halo)
    wupT = nc.dram_tensor("wupT", [D, 2 * INNER], BF16, kind="ExternalInput")  # W_up.T
    wqkvT = nc.dram_tensor("wqkvT", [INNER, 3 * INNER], BF16, kind="ExternalInput")
    wgT = nc.dram_tensor("wgT", [INNER, 2 * NH], BF16, kind="ExternalInput")   # [Wig.T | Wfg.T]
    convw = nc.dram_tensor("convw", [DH, (INNER // DH) * KCONV], F32, kind="ExternalInput")
    convb = nc.dram_tensor("convb", [DH, INNER // DH], F32, kind="ExternalInput")
    skip_i = nc.dram_tensor("skip_i", [DH, INNER // DH], F32, kind="ExternalInput")

    q_o = nc.dram_tensor("q_o", [INNER, TOK], BF16, kind="ExternalOutput")
    k_o = nc.dram_tensor("k_o", [INNER, TOK], BF16, kind="ExternalOutput")
    v_o = nc.dram_tensor("v_o", [INNER, TOK], BF16, kind="ExternalOutput")
    sxm_o = nc.dram_tensor("sxm_o", [INNER, TOK], BF16, kind="ExternalOutput")   # skip * xm
    g2_o = nc.dram_tensor("g2_o", [INNER, TOK], BF16, kind="ExternalOutput")     # silu(x_og)
    gates_o = nc.dram_tensor("gates_o", [2 * NH, TOK], F32, kind="ExternalOutput")

    KT_UP = D // DH          # 4 k-tiles for up-proj
    FT = INNER // DH         # 8 feature tiles of the mlstm half
    KT_IN = INNER // DH      # 8 k-tiles over INNER
    MT_QKV = 3 * FT          # 24
    HALO = KCONV - 1         # 3

    with tile.TileContext(nc) as tc, \
         tc.tile_pool(name="const", bufs=1) as const, \
         tc.tile_pool(name="big", bufs=1) as big, \
         tc.tile_pool(name="ev", bufs=3) as ev, \
         tc.tile_pool(name="wpool", bufs=4) as wpool, \
         tc.tile_pool(name="gev", bufs=1) as gev, \
         tc.tile_pool(name="cv", bufs=3) as cv, \
         tc.tile_pool(name="ps", bufs=2, space="PSUM") as ps:
        if True:
            # --- load weights / x interleaved in PE consumption order:
            # wup m-slices and xt token-chunks arrive just ahead of the
            # up-projection chains that need them.
            wup_sb = big.tile([DH, KT_UP, 2 * INNER], BF16)
            xt_sb = big.tile([DH, KT_UP, TH], BF16)
            XCH = [(0, 515), (515, 512), (1027, 512), (1539, 512)]
            # PE m-tile order for the up-proj (mlstm front-loaded, og fills)
            UP_ORDER = [0, 1, 2, 8, 3, 9, 4, 10, 5, 11, 6, 12, 7, 13, 14, 15]

            def wup_slice(j):
                nc.sync.dma_start(
                    out=wup_sb[:, :, j * DH:(j + 1) * DH],
                    in_=wupT[:, j * DH:(j + 1) * DH].rearrange("(kt p) m -> p kt m", p=DH))
            for j in UP_ORDER[:4]:
                wup_slice(j)
            for c, (c0, cn) in enumerate(XCH):
                for kt in range(KT_UP):
                    nc.sync.dma_start(out=xt_sb[:, kt, c0:c0 + cn],
                                      in_=xt[kt * DH:(kt + 1) * DH, c0:c0 + cn])
            for j in UP_ORDER[4:]:
                wup_slice(j)
            convw_sb = const.tile([DH, FT, KCONV], F32)
            nc.sync.dma_start(out=convw_sb, in_=convw[:].rearrange("p (ft t) -> p ft t", ft=FT))
            convb_sb = const.tile([DH, FT], F32)
            nc.sync.dma_start(out=convb_sb, in_=convb[:])
            skip_sb = const.tile([DH, FT], F32)
            nc.sync.dma_start(out=skip_sb, in_=skip_i[:])
            wg_sb = const.tile([DH, KT_IN, 2 * NH], BF16)
            nc.sync.dma_start(out=wg_sb, in_=wgT[:].rearrange("(kt p) m -> p kt m", p=DH))

            xpre_sb = big.tile([DH, FT, TH], BF16)   # [p, ft, halo+tok]
            xm_sb = big.tile([DH, FT, TOK], BF16)
            xog_sb = big.tile([DH, FT, TOK], BF16)

            # --- halo chains (tokens 0..2 of xpre), one psum tile + one evac
            pt_h = ps.tile([DH, FT, HALO], F32, tag="mm")
            for m in range(FT):
                for kt in range(KT_UP):
                    nc.tensor.matmul(
                        pt_h[:, m, :],
                        wup_sb[:, kt, m * DH:(m + 1) * DH],
                        xt_sb[:, kt, 0:HALO],
                        start=(kt == 0), stop=(kt == KT_UP - 1))
            nc.scalar.copy(xpre_sb[:, :, 0:HALO], pt_h[:])

            rot = [0]
            def evac(dst, src, eng=None):
                # PSUM sources: only ACT/DVE may read PSUM (not GPSIMD)
                if eng is None:
                    eng = 'ad'[rot[0] % 2]
                    rot[0] += 1
                if eng == 'a':
                    nc.scalar.copy(dst, src)
                else:
                    nc.vector.tensor_copy(dst, src)

            # conv per ft: taps 0/2 as DVE tensor_scalar (4x mode), tap 1 on
            # Pool / tap 3 on DVE scalar_tensor_tensor, final add on DVE,
            # sigmoid on ACT, silu-mult on Pool.
            def emit_conv(ft):
                def xs(tau):
                    return xpre_sb[:, ft, tau:tau + TOK]
                # taps on DVE tensor_scalar (4x mode); conv bias folded into
                # tap 0; adds split DVE/Pool; silu-mult on Pool (SBUF only)
                p0 = cv.tile([DH, TOK], BF16, tag="ca")
                nc.vector.tensor_scalar(p0[:], xs(0), convw_sb[:, ft, 0:1],
                                        convb_sb[:, ft:ft + 1], OP.mult, OP.add)
                p1 = cv.tile([DH, TOK], BF16, tag="cb")
                nc.vector.tensor_scalar_mul(p1[:], xs(1), convw_sb[:, ft, 1:2])
                p2 = cv.tile([DH, TOK], BF16, tag="ca")
                nc.vector.tensor_scalar_mul(p2[:], xs(2), convw_sb[:, ft, 2:3])
                p3 = cv.tile([DH, TOK], BF16, tag="cb")
                nc.vector.tensor_scalar_mul(p3[:], xs(3), convw_sb[:, ft, 3:4])
                q0 = cv.tile([DH, TOK], BF16, tag="ca")
                nc.gpsimd.tensor_tensor(q0[:], p0[:], p1[:], OP.add)
                q1 = cv.tile([DH, TOK], BF16, tag="cb")
                nc.vector.tensor_tensor(q1[:], p2[:], p3[:], OP.add)
                y = cv.tile([DH, TOK], BF16, tag="y", bufs=2)
                nc.vector.tensor_tensor(y[:], q0[:], q1[:], OP.add)
                sg = cv.tile([DH, TOK], BF16, tag="sg", bufs=2)
                nc.scalar.activation(sg[:], y[:], AF.Sigmoid)
                nc.gpsimd.tensor_tensor(xm_sb[:, ft, :], y[:], sg[:], OP.mult)
                # sxm = skip * xm (DVE 4x)
                sxm_t = ev.tile([DH, TOK], BF16, tag="out")
                nc.vector.tensor_scalar_mul(sxm_t[:], xm_sb[:, ft, :],
                                            skip_sb[:, ft:ft + 1])
                nc.sync.dma_start(out=sxm_o[ft * DH:(ft + 1) * DH, :], in_=sxm_t[:])

            # --- up-projection in UP_ORDER (j<8: mlstm m-tile j, then its
            # conv; j>=8: og m-tile j-8, single-copy evac)
            for j in UP_ORDER:
                pt = ps.tile([DH, 4, 512], F32, tag="mm")
                for ns in range(4):
                    for kt in range(KT_UP):
                        nc.tensor.matmul(
                            pt[:, ns, :],
                            wup_sb[:, kt, j * DH:(j + 1) * DH],
                            xt_sb[:, kt, HALO + ns * 512: HALO + (ns + 1) * 512],
                            start=(kt == 0), stop=(kt == KT_UP - 1))
                if j < FT:
                    evac(xpre_sb[:, j, HALO:HALO + TOK],
                         pt[:].rearrange("p a b -> p (a b)"), eng='a')
                    emit_conv(j)
                else:
                    evac(xog_sb[:, j - FT, :], pt[:].rearrange("p a b -> p (a b)"),
                         eng='a' if (j % 2) else 'd')

            # --- g2 = silu(x_og) from SBUF (runs during gates/qkv PE work)
            for m in range(FT):
                sg2 = cv.tile([DH, TOK], BF16, tag="sg", bufs=2)
                nc.scalar.activation(sg2[:], xog_sb[:, m, :], AF.Sigmoid)
                g2_t = ev.tile([DH, TOK], BF16, tag="out")
                if m % 2 == 0:
                    nc.gpsimd.tensor_tensor(g2_t[:], xog_sb[:, m, :], sg2[:], OP.mult)
                else:
                    nc.vector.tensor_tensor(g2_t[:], xog_sb[:, m, :], sg2[:], OP.mult)
                nc.sync.dma_start(out=g2_o[m * DH:(m + 1) * DH, :], in_=g2_t[:])

            # --- gates: [16, TOK] f32
            ptg = ps.tile([2 * NH, 4, 512], F32, tag="mm")
            for ns in range(4):
                for kt in range(KT_IN):
                    nc.tensor.matmul(
                        ptg[:, ns, :], wg_sb[:, kt, :],
                        xm_sb[:, kt, ns * 512:(ns + 1) * 512],
                        start=(kt == 0), stop=(kt == KT_IN - 1))
            for hf in range(2):
                gv = gev.tile([2 * NH, TOK // 2], F32, tag="gv")
                nc.vector.tensor_copy(
                    gv[:], ptg[:, hf * 2:(hf + 1) * 2, :].rearrange("p a b -> p (a b)"))
                nc.sync.dma_start(
                    out=gates_o[:, hf * (TOK // 2):(hf + 1) * (TOK // 2)], in_=gv[:])

            # --- q/k/v projections (streamed weights)
            qkv_outs = [q_o, k_o, v_o]
            for m in range(MT_QKV):
                # weights streamed on the ACT queue so they never wait
                # behind the big output DMAs on SP
                w_sb = wpool.tile([DH, KT_IN, DH], BF16, tag="w")
                nc.sync.dma_start(
                    out=w_sb,
                    in_=wqkvT[:, m * DH:(m + 1) * DH].rearrange("(kt p) m -> p kt m", p=DH))
                out_t = qkv_outs[m // FT]
                mf = m % FT
                pt = ps.tile([DH, 4, 512], F32, tag="mm")
                for ns in range(4):
                    for kt in range(KT_IN):
                        nc.tensor.matmul(
                            pt[:, ns, :], w_sb[:, kt, :],
                            xm_sb[:, kt, ns * 512:(ns + 1) * 512],
                            start=(kt == 0), stop=(kt == KT_IN - 1))
                ev_t = ev.tile([DH, TOK], BF16, tag="ev")
                evac(ev_t[:], pt[:].rearrange("p a b -> p (a b)"))
                nc.sync.dma_start(out=out_t[mf * DH:(mf + 1) * DH, :], in_=ev_t[:])
    nc.compile()
    return nc


# ---------------------------------------------------------------- phase B ----
def build_phase_b():
    nc = new_nc()
    ins = {}
    outs = {}
    for u in range(NUNIT):
        # feat-major q and (esc*cch)-scaled k
        ins[f"qT{u}"] = nc.dram_tensor(f"qT{u}", [DH, S], BF16, kind="ExternalInput")
        ins[f"kTc{u}"] = nc.dram_tensor(f"kTc{u}", [DH, S], BF16, kind="ExternalInput")
        # token-major (p = token-in-chunk): [p, c, d] esc-scaled k, [p, c, e] = [v | 1]
        ins[f"kesc{u}"] = nc.dram_tensor(f"kesc{u}", [DH, NCH * DH], BF16, kind="ExternalInput")
        ins[f"vone{u}"] = nc.dram_tensor(f"vone{u}", [DH, NCH * NW], BF16, kind="ExternalInput")
        # packed per-unit scalars: [p, {dec(col0=0), e2, e3}, NCH] f32
        ins[f"scal{u}"] = nc.dram_tensor(f"scal{u}", [DH, 3 * NCH], F32, kind="ExternalInput")
        # h out in [p, c, e] layout
        outs[f"h{u}"] = nc.dram_tensor(f"h{u}", [DH, NCH * DH], BF16, kind="ExternalOutput")
    mask_i = nc.dram_tensor("mask_i", [DH, 4 * DH], BF16, kind="ExternalInput")
    BDBG = bool(os.environ.get("BDBG"))
    if BDBG:
        for nm in ("dbgU0", "dbgCs0", "dbgdecf0", "dbgsp0", "dbgdraw0"):
            sz = NW * NCH if nm not in ("dbgsp0", "dbgdraw0") else 4 * DH * 4
            outs[nm] = nc.dram_tensor(nm, [DH, sz], BF16, kind="ExternalOutput")

    SB = 4          # chunks per block
    GRP = 4         # blocks per den/h-DMA group
    NE = NW * NCH   # 8256 elements per big ring buffer
    ESPLIT = 65     # scan e-split: DVE does e<65, Pool e>=65
    with tile.TileContext(nc) as tc, \
         tc.tile_pool(name="small", bufs=1) as small, \
         tc.tile_pool(name="sh", bufs=10) as sh, \
         tc.tile_pool(name="spb", bufs=3) as spb, \
         tc.tile_pool(name="hrb", bufs=4) as hrb, \
         tc.tile_pool(name="hgo", bufs=2) as hgo, \
         tc.tile_pool(name="den", bufs=2) as den, \
         tc.tile_pool(name="ps1", bufs=2, space="PSUM") as ps1, \
         tc.tile_pool(name="psn", bufs=3, space="PSUM") as psn:
        if True:
            def ring(name):
                return sh.tile([DH, NE], BF16, tag="sh", name=name)

            mask_sb = small.tile([DH, SB, DH], BF16, name="mask")
            nc.sync.dma_start(
                out=mask_sb, in_=mask_i[:].rearrange("p (b l) -> p b l", b=SB))
            T = {}
            for u in range(NUNIT):
                T[u] = dict(scal=small.tile([DH, 3, NCH], F32, name=f"scal{u}"))
                nc.sync.dma_start(
                    out=T[u]['scal'],
                    in_=ins[f"scal{u}"][:].rearrange("p (k c) -> p k c", k=3))
            # ring allocation order (11 bufs): kesc1 (12th) wraps onto kesc0,
            # which dies after stage_i(0).
            T[0]['kesc'] = ring("kesc0")
            T[0]['U'] = ring("U0")
            T[0]['vone'] = ring("vone0")
            T[0]['Cs'] = ring("Cs0")
            T[0]['qT'] = ring("qT0")
            T[0]['kTc'] = ring("kTc0")
            T[1]['vone'] = ring("vone1")
            T[1]['U'] = ring("U1")
            T[1]['Cs'] = ring("Cs1")
            T[1]['qT'] = ring("qT1")
            # wraps (11th/12th allocs -> slots 1/2): kesc1 -> kesc0 (dead
            # after stage_i(0)); kTc1 -> U0 (dead after scan(0))
            T[1]['kesc'] = ring("kesc1")
            T[1]['kTc'] = ring("kTc1")
            # decay tile shared by both units (rebuilt between scans)
            decf_sh = small.tile([DH, NE], BF16, name="decf_sh")
            T[0]['decf'] = decf_sh
            T[1]['decf'] = decf_sh

            HALF = NCH // 2
            def issue_unit_dmas(u):
                # all inputs on SP: u1's ring-slot waits only delay u1
                # issues, which are transfer-bound anyway
                eng = nc.sync
                for half in range(2):
                    ks = slice(half * HALF * DH, (half + 1) * HALF * DH)
                    vs = slice(half * HALF * NW, (half + 1) * HALF * NW)
                    eng.dma_start(out=T[u]['kesc'][:, ks],
                                  in_=ins[f"kesc{u}"][:, ks])
                    eng.dma_start(out=T[u]['vone'][:, vs],
                                  in_=ins[f"vone{u}"][:, vs])
                order = ('qT', 'kTc') if u == 0 else ('kTc', 'qT')
                for half in range(2):
                    ts = slice(half * (S // 2), (half + 1) * (S // 2))
                    for nm in order:
                        eng.dma_start(out=T[u][nm][:, ts],
                                      in_=ins[f"{nm}{u}"][:, ts])

            def kescv(u):
                return T[u]['kesc'][:, :NCH * DH].rearrange("p (c d) -> p c d", c=NCH)

            def vonev(u):
                return T[u]['vone'][:].rearrange("p (c e) -> p c e", c=NCH)

            def Uv(u):
                return T[u]['U'][:].rearrange("p (e c) -> p e c", e=NW)

            def Csv(u):
                return T[u]['Cs'][:].rearrange("p (e c) -> p e c", e=NW)

            def stage_i(u, half):
                for cb in range(half * (HALF // SB), (half + 1) * (HALF // SB)):
                    up = psn.tile([DH, SB, 256], F32, tag="nps")
                    for i in range(SB):
                        c = cb * SB + i
                        nc.tensor.matmul(
                            up[:, i, :NW], kescv(u)[:, c, :], vonev(u)[:, c, :],
                            start=True, stop=True)
                    # PSUM -> SBUF bf16, strided into [p, e, c] layout
                    dst = Uv(u)[:, :, cb * SB:(cb + 1) * SB].rearrange("p e c -> p c e")
                    nc.scalar.copy(dst, up[:, :, :NW])

            def build_decf(u):
                # decf[p, (e, c)] = dec[p, c]; col c=0 is 0 => resets the
                # carried state at each e-boundary during the scan.
                t = T[u]
                dec_bc = t['scal'][:, 0, :]
                c0 = ESPLIT * NCH
                nc.vector.tensor_copy(
                    t['decf'][:, :c0].rearrange("p (e c) -> p e c", e=ESPLIT),
                    dec_bc.unsqueeze(1).broadcast_to([DH, ESPLIT, NCH]))
                nc.gpsimd.tensor_copy(
                    t['decf'][:, c0:].rearrange("p (e c) -> p e c", e=NW - ESPLIT),
                    dec_bc.unsqueeze(1).broadcast_to([DH, NW - ESPLIT, NCH]))

            def scan(u):
                # scans are DVE-only on HW (TensorScalarPtr engine check)
                t = T[u]
                nc.vector.tensor_tensor_scan(
                    t['Cs'][:], t['decf'][:], t['U'][:],
                    0.0, OP.mult, OP.add)

            def stage_ii_group(u, g):
                t = T[u]
                c0 = g * GRP * SB                 # first chunk of group
                draw = den.tile([DH, GRP * SB], F32, tag="draw")
                hrs = []
                for blk in range(GRP):
                    cb = g * GRP + blk
                    sps = ps1.tile([DH, SB, DH], F32, tag="sps")
                    for i in range(SB):
                        c = cb * SB + i
                        csl = slice(c * LC, (c + 1) * LC)
                        nc.tensor.matmul(
                            sps[:, i, :], t['kTc'][:, csl], t['qT'][:, csl],
                            start=True, stop=True)
                    # Sp = S * mask (esc/cch scaling baked into kTc)
                    sp = spb.tile([DH, SB, DH], BF16, tag="sp")
                    nc.vector.tensor_tensor(sp[:], sps[:], mask_sb[:], OP.mult)
                    # num matmuls
                    if BDBG and u == 0 and g == 0 and blk == 0:
                        nc.sync.dma_start(out=outs["dbgsp0"][:, :SB * DH].rearrange("p (a b) -> p a b", a=SB), in_=sp[:])
                    nps = psn.tile([DH, SB, 256], F32, tag="nps")
                    for i in range(SB):
                        c = cb * SB + i
                        csl = slice(c * LC, (c + 1) * LC)
                        if c > 0:
                            nc.tensor.matmul(
                                nps[:, i, :NW], t['qT'][:, csl], Csv(u)[:, :, c - 1],
                                start=True, stop=False)
                        nc.tensor.matmul(
                            nps[:, i, :NW], sp[:, i, :], vonev(u)[:, c, :],
                            start=(c == 0), stop=True)
                    # raw den column (f32) + unscaled h (bf16) out of PSUM
                    nc.scalar.copy(
                        draw[:, blk * SB:(blk + 1) * SB], nps[:, :, DH])
                    hr = hrb.tile([DH, SB, DH], BF16, tag="hr")
                    if blk % 2 == 0:
                        nc.scalar.copy(hr[:], nps[:, :, :DH])
                    else:
                        nc.vector.tensor_copy(hr[:], nps[:, :, :DH])
                    hrs.append(hr)
                if BDBG and u == 0 and g == 0:
                    nc.gpsimd.dma_start(out=outs["dbgdraw0"][:, :GRP * SB],
                                      in_=draw[:])
                # batched den for the 16-chunk group: den = max(|raw|, e2) + e3
                gsl = slice(c0, c0 + GRP * SB)
                dabs = den.tile([DH, GRP * SB], F32, tag="dabs")
                nc.scalar.activation(dabs[:], draw[:], AF.Abs)
                dmx = den.tile([DH, GRP * SB], F32, tag="dmx")
                nc.vector.tensor_tensor(dmx[:], dabs[:], t['scal'][:, 1, gsl], OP.max)
                dpl = den.tile([DH, GRP * SB], F32, tag="dpl")
                nc.vector.tensor_tensor(dpl[:], dmx[:], t['scal'][:, 2, gsl], OP.add)
                rden = den.tile([DH, GRP * SB], F32, tag="rden")
                nc.vector.reciprocal(rden[:], dpl[:])
                # h = hr * rden (broadcast over d), batched out-DMA per group
                hg = hgo.tile([DH, GRP * SB, DH], BF16, tag="hg")
                for blk in range(GRP):
                    bsl = slice(blk * SB, (blk + 1) * SB)
                    rb = rden[:, bsl].unsqueeze(2).broadcast_to([DH, SB, DH])
                    nc.gpsimd.tensor_tensor(hg[:, bsl, :], hrs[blk][:], rb, OP.mult)
                nc.sync.dma_start(
                    out=outs[f"h{u}"][:, c0 * DH:(c0 + GRP * SB) * DH], in_=hg[:])

            issue_unit_dmas(0)
            issue_unit_dmas(1)
            with tc.high_priority():
                build_decf(0)
                stage_i(0, 0)
                stage_i(0, 1)
                scan(0)
            if BDBG:
                for nm, t_ in (("dbgU0", T[0]['U']), ("dbgCs0", T[0]['Cs']),
                               ("dbgdecf0", T[0]['decf'])):
                    nc.sync.dma_start(out=outs[nm][:], in_=t_[:])
            with tc.tile_wait_until(0.021):
                stage_ii_group(0, 0)
            with tc.tile_wait_until(0.025):
                stage_ii_group(0, 1)
            with tc.tile_wait_until(0.031):
                stage_i(1, 0)
            with tc.tile_wait_until(0.033):
                stage_ii_group(0, 2)
            with tc.tile_wait_until(0.037):
                stage_i(1, 1)
                build_decf(1)
                scan(1)
            with tc.tile_wait_until(0.041):
                stage_ii_group(0, 3)
            for g in range(NCH // (GRP * SB)):
                with tc.tile_wait_until(0.046 + 0.005 * g):
                    stage_ii_group(1, g)
    nc.compile()
    return nc


# ---------------------------------------------------------------- phase C ----
def build_phase_c():
    nc = new_nc()
    hg_i = nc.dram_tensor("hg_i", [INNER, TOK], BF16, kind="ExternalInput")  # (h+skip*xm)*silu(x_og)
    wdT = nc.dram_tensor("wdT", [INNER, D], BF16, kind="ExternalInput")
    out_o = nc.dram_tensor("out_o", [D, TOK], BF16, kind="ExternalOutput")

    FT = INNER // DH   # 8
    MT = D // DH       # 4
    NS = TOK // 512    # 4 token slices
    with tile.TileContext(nc) as tc, \
         tc.tile_pool(name="big", bufs=1) as big, \
         tc.tile_pool(name="ev", bufs=4) as ev, \
         tc.tile_pool(name="ps", bufs=4, space="PSUM") as ps:
        if True:
            wd_sb = big.tile([DH, FT, D], BF16)
            hg_sb = big.tile([DH, FT, TOK], BF16)
            # weight m-tile 0 first so the first chain starts early
            nc.sync.dma_start(out=wd_sb[:, :, :DH],
                              in_=wdT[:, :DH].rearrange("(ft p) m -> p ft m", p=DH))
            for ns in range(NS):
                tsl = slice(ns * 512, (ns + 1) * 512)
                nc.sync.dma_start(
                    out=hg_sb[:, :, tsl],
                    in_=hg_i[:, tsl].rearrange("(ft p) t -> p ft t", p=DH))
                if ns == 0:
                    nc.sync.dma_start(
                        out=wd_sb[:, :, DH:],
                        in_=wdT[:, DH:].rearrange("(ft p) m -> p ft m", p=DH))
            for ns in range(NS):
                tsl = slice(ns * 512, (ns + 1) * 512)
                for m in range(MT):
                    pt = ps.tile([DH, 512], F32)
                    for kt in range(FT):
                        nc.tensor.matmul(
                            pt[:], wd_sb[:, kt, m * DH:(m + 1) * DH],
                            hg_sb[:, kt, tsl],
                            start=(kt == 0), stop=(kt == FT - 1))
                    ot = ev.tile([DH, 512], BF16, tag="ot")
                    if m % 2 == 0:
                        nc.scalar.copy(ot[:], pt[:])
                    else:
                        nc.vector.tensor_copy(ot[:], pt[:])
                    nc.sync.dma_start(
                        out=out_o[m * DH:(m + 1) * DH, tsl], in_=ot[:])
    nc.compile()
    return nc


# ------------------------------------------------------------- host glue ----
def host_gate_math(i_pre, f_pre):
    """i_pre, f_pre: [B, NH, S] f32.  Returns dict of f32 arrays.

    Exports (per b, h):
      esc   [S]        exp(a_j - m_new(chunk))            per-token k scale
      cch   [NCH]      exp(m_new - ms - scaG)             per-chunk Sp scale
      dec   [NCH]      exp(scaG + ms - m_new), col0 = 0   scan multiplier
      e2,e3 [NCH, LC]  den clamp terms (qk_scale folded)
    """
    i_pre = i_pre.astype(np.float64)
    f_pre = f_pre.astype(np.float64)
    vecI = np.log(1.0 / (1.0 + np.exp(-i_pre)) + EPS)
    vecF = np.log(1.0 / (1.0 + np.exp(-f_pre)) + EPS)
    Ic = vecI.reshape(B, NH, NCH, LC)
    Fc = vecF.reshape(B, NH, NCH, LC)
    vecB = np.cumsum(Fc, axis=-1)
    scaG = vecB[..., -1]
    vecA = scaG[..., None] - vecB + Ic

    ms = np.zeros((B, NH, NCH))
    dec = np.zeros((B, NH, NCH))
    m_new_arr = np.zeros((B, NH, NCH))
    m = np.zeros((B, NH))
    for c in range(NCH):
        amax = vecA[:, :, c, :].max(-1)
        m_new = np.maximum(scaG[:, :, c] + m, amax)
        ms[:, :, c] = m
        dec[:, :, c] = np.exp(scaG[:, :, c] + m - m_new)
        m_new_arr[:, :, c] = m_new
        m = m_new
    escale = np.exp(vecA - m_new_arr[..., None])          # [B,NH,NCH,LC]
    cch = np.exp(m_new_arr - ms - scaG)                   # [B,NH,NCH]

    mask = np.tril(np.ones((LC, LC), bool))
    logD = vecB[..., :, None] - vecB[..., None, :] + Ic[..., None, :]
    logD = np.where(mask, logD, -np.inf)
    m_intra = logD.max(-1)
    m_comb = np.maximum(vecB + ms[..., None], m_intra)
    e2 = np.exp(-vecB - ms[..., None]) / QK_SCALE
    e3 = EPS * np.exp(m_comb - vecB - ms[..., None]) / QK_SCALE
    dec0 = dec.copy()
    dec0[:, :, 0] = 0.0
    return dict(
        esc=escale.reshape(B, NH, S).astype(np.float32),
        cch=cch.astype(np.float32),
        dec=dec0.astype(np.float32),
        e2=e2.astype(np.float32), e3=e3.astype(np.float32))


def prep_weights(W_up, Wq, Wk, Wv, W_ig, W_fg, conv_w, conv_b, skip, W_down):
    """Host-side weight packing (same for all cores)."""
    FT = INNER // DH
    wupT = _bf(W_up.T)                                         # [512, 2048]
    wqkvT = _bf(np.concatenate([Wq.T, Wk.T, Wv.T], axis=1))    # [1024, 3072]
    wgT = _bf(np.concatenate([W_ig.T, W_fg.T], axis=1))        # [1024, 16]
    convw = np.ascontiguousarray(
        conv_w.reshape(FT, DH, KCONV).transpose(1, 0, 2).reshape(DH, FT * KCONV)
    ).astype(np.float32)
    convb = np.ascontiguousarray(conv_b.reshape(FT, DH).T).astype(np.float32)
    skip_p = np.ascontiguousarray(skip.reshape(FT, DH).T).astype(np.float32)
    wdT = _bf(W_down.T)                                        # [1024, 512]
    mask4 = _bf(np.tile(np.tril(np.ones((DH, DH), np.float32)), (1, 4)))
    return dict(wupT=wupT, wqkvT=wqkvT, wgT=wgT, convw=convw, convb=convb,
                skip_p=skip_p, wdT=wdT, mask4=mask4)


def build_a_inmaps(x, wp):
    """Per-core phase A input maps.  Core c = (b=c//4, quarter=c%4)."""
    in_maps = []
    for c in range(8):
        b, qt = c // 4, c % 4
        s0 = qt * TOK
        xs = x[b, :, :].T                                       # [512, S] view
        if s0 == 0:
            xt = np.concatenate([np.zeros((D, KCONV - 1), np.float32),
                                 xs[:, :TOK]], axis=1)
        else:
            xt = xs[:, s0 - (KCONV - 1): s0 + TOK]
        in_maps.append(dict(
            xt=_bf(xt), wupT=wp['wupT'], wqkvT=wp['wqkvT'], wgT=wp['wgT'],
            convw=wp['convw'], convb=wp['convb'], skip_i=wp['skip_p']))
    return in_maps


def assemble_a_outputs(a_results, b_ig, b_fg):
    """Concatenate per-core phase A outputs into full feature-major tensors."""
    def cat(name):
        return np.stack([
            np.concatenate([a_results[b * 4 + qt][name] for qt in range(4)], axis=1)
            for b in range(B)])
    q_t, k_t, v_t = cat('q_o'), cat('k_o'), cat('v_o')          # [B, INNER, S] bf16
    sxm_t, g2_t = cat('sxm_o'), cat('g2_o')
    gates = cat('gates_o').astype(np.float32)                   # [B, 16, S]
    i_pre = gates[:, :NH, :] + np.asarray(b_ig, np.float32)[None, :, None]
    f_pre = gates[:, NH:, :] + np.asarray(b_fg, np.float32)[None, :, None]
    return q_t, k_t, v_t, sxm_t, g2_t, i_pre, f_pre


def build_b_inmaps(q_t, k_t, v_t, g, wp):
    """Per-core phase B inputs.  Core c handles units (b, 2h) where
    b = c // 4, heads (2*(c%4), 2*(c%4)+1)."""
    in_maps = []
    for c in range(8):
        b, hp = c // 4, c % 4
        m = {'mask_i': wp['mask4']}
        for u in range(NUNIT):
            h = 2 * hp + u
            rs = slice(h * DH, (h + 1) * DH)
            kf = k_t[b, rs, :].astype(np.float32)               # [128, S] feat-major
            esc = g['esc'][b, h]                                # [S]
            cch_tok = np.repeat(g['cch'][b, h], LC)             # [S]
            m[f"qT{u}"] = np.ascontiguousarray(q_t[b, rs, :])
            m[f"kTc{u}"] = _bf(kf * (esc * cch_tok)[None, :])
            # token-major [p, c, d] / [p, c, e]
            kesc = (kf.T * esc[:, None]).reshape(NCH, LC, DH)
            m[f"kesc{u}"] = _bf(kesc.transpose(1, 0, 2).reshape(DH, NCH * DH))
            vone = np.empty((NCH, LC, NW), np.float32)
            vone[:, :, :DH] = v_t[b, rs, :].astype(np.float32).T.reshape(NCH, LC, DH)
            vone[:, :, DH] = 1.0
            m[f"vone{u}"] = _bf(vone.transpose(1, 0, 2).reshape(DH, NCH * NW))
            scal = np.empty((DH, 3, NCH), np.float32)
            scal[:, 0, :] = g['dec'][b, h][None, :]
            scal[:, 1, :] = g['e2'][b, h].T
            scal[:, 2, :] = g['e3'][b, h].T
            m[f"scal{u}"] = np.ascontiguousarray(scal.reshape(DH, 3 * NCH))
        in_maps.append(m)
    return in_maps


def build_c_inmaps(b_results, sxm_t, g2_t, wp):
    """Assemble h from phase B [p, c, e] layouts into feature-major h_t,
    fold the gating (hg = (h + skip*xm) * silu(x_og)) during the
    reshard, then per-core phase C inputs."""
    h_t = np.empty((B, INNER, S), dtype=np.float32)
    for c in range(8):
        b, hp = c // 4, c % 4
        for u in range(NUNIT):
            h = 2 * hp + u
            # [p, c, e] -> [e, c*LC + p]
            harr = np.asarray(b_results[c][f"h{u}"]).astype(np.float32)
            harr = harr.reshape(DH, NCH, DH)
            h_t[b, h * DH:(h + 1) * DH, :] = (
                harr.transpose(2, 1, 0).reshape(DH, S))
    hg_t = _bf((h_t + np.asarray(sxm_t, np.float32))
               * np.asarray(g2_t, np.float32))
    in_maps = []
    for c in range(8):
        b, qt = c // 4, c % 4
        ts = slice(qt * TOK, (qt + 1) * TOK)
        in_maps.append(dict(
            hg_i=np.ascontiguousarray(hg_t[b, :, ts]),
            wdT=wp['wdT']))
    return in_maps


def assemble_output(c_results):
    out = np.empty((B, S, D), np.float32)
    for c in range(8):
        b, qt = c // 4, c % 4
        out[b, qt * TOK:(qt + 1) * TOK, :] = \
            np.asarray(c_results[c]['out_o']).astype(np.float32).T
    return out


# ------------------------------------------------------------------ entry ----
from concourse.bass_utils import run_bass_kernel_spmd as _run_spmd

_CACHE = {}


def _programs():
    if 'a' not in _CACHE:
        _CACHE['a'] = build_phase_a()
        _CACHE['b'] = build_phase_b()
        _CACHE['c'] = build_phase_c()
    return _CACHE['a'], _CACHE['b'], _CACHE['c']


def kernel(x, W_up, Wq, Wk, Wv, W_ig, b_ig, W_fg, b_fg, conv_w, conv_b, skip,
           W_down):
    x = np.asarray(x, np.float32)
    nc_a, nc_b, nc_c = _programs()
    cores = list(range(8))
    wp = prep_weights(W_up, Wq, Wk, Wv, W_ig, W_fg, conv_w, conv_b, skip, W_down)
    a_maps = build_a_inmaps(x, wp)
    ra = _run_spmd(nc_a, a_maps, core_ids=cores).results
    q_t, k_t, v_t, sxm_t, g2_t, i_pre, f_pre = assemble_a_outputs(ra, b_ig, b_fg)
    g = host_gate_math(i_pre, f_pre)
    b_maps = build_b_inmaps(q_t, k_t, v_t, g, wp)
    rb = _run_spmd(nc_b, b_maps, core_ids=cores).results
    c_maps = build_c_inmaps(rb, sxm_t, g2_t, wp)
    rc = _run_spmd(nc_c, c_maps, core_ids=cores).results
    return assemble_output(rc)


# revision 44
# speedup vs baseline: 1.0079x; 1.0079x over previous
"""Trainium2 Bass kernel for nn_ChunkwiseMLSTM (B=2, S=8192, D=512, INNER=1024, NH=8).

kernel(**inputs) -> np.ndarray [2, 8192, 512] f32.

Three SPMD launches on 8 NeuronCores:
  A: token-sharded projections (up-proj, causal conv+SiLU split across
     DVE/Pool with 4x tensor_scalar taps, q/k/v, gate pre-acts, skip*xm,
     silu(x_og) fused into single ACT ops straight from PSUM).
  B: head-sharded chunkwise mLSTM (LC=128 chunks):
     - stage i: inter-chunk state updates U_c = kesc_c^T @ [v|1] (PE),
       strided PSUM->SBUF evacs (ACT/Pool), then per-unit e-split
       tensor_tensor_scan on DVE+Pool with broadcast-AP decay input.
     - stage ii: per 4-chunk blocks: S matmuls (PE), mask-mult, num
       matmuls, f32 den column extraction; den clamp math batched per
       16-chunk group; h scaling + group-batched DMA out (Pool queue).
  C: token-sharded output gating + down-projection, 512-token-sliced
     streaming; host pre-folds h+skip*xm during the B->C reshard.
Host between launches: gate-derived scan scalars (O(B*NH*S)), weight
pre-transposition, resharding.
"""
import os
os.environ.setdefault("JAX_COMPILATION_CACHE_DIR",
                      os.path.expanduser("~/.cache/jax_bass_cache"))
os.environ.setdefault("JAX_PERSISTENT_CACHE_MIN_ENTRY_SIZE_BYTES", "0")
os.environ.setdefault("JAX_PERSISTENT_CACHE_MIN_COMPILE_TIME_SECS", "0")

import sys
if '/opt/trn_rl_repo' not in sys.path:
    sys.path.insert(0, '/opt/trn_rl_repo')

import numpy as np
import ml_dtypes

import concourse.bass as bass
import concourse.tile as tile
from concourse import mybir, bacc

F32 = mybir.dt.float32
BF16 = mybir.dt.bfloat16
AF = mybir.ActivationFunctionType
OP = mybir.AluOpType

B, S, D = 2, 8192, 512
INNER, NH, KCONV = 1024, 8, 4
DH = 128
EPS = 1e-6
LC = 128           # chunk length used on device (math is chunk-size invariant)
NCH = S // LC      # 64
NW = 129           # [C | n] state width
QK_SCALE = DH ** -0.5
TOK = S // 4       # tokens per core in phases A/C = 2048
TH = TOK + (KCONV - 1)   # 2051 with conv halo
NUNIT = 2          # (b,h) units per core in phase B


def _bf(x):
    return np.ascontiguousarray(np.asarray(x, np.float32).astype(ml_dtypes.bfloat16))


def new_nc():
    return bacc.Bacc(None, target_bir_lowering=False, debug=False)


# ---------------------------------------------------------------- phase A ----
def build_phase_a():
    nc = new_nc()
    xt = nc.dram_tensor("xt", [D, TH], BF16, kind="ExternalInput")            # x[b].T slice (halo)
    wupT = nc.dram_tensor("wupT", [D, 2 * INNER], BF16, kind="ExternalInput")  # W_up.T
    wqkvT = nc.dram_tensor("wqkvT", [INNER, 3 * INNER], BF16, kind="ExternalInput")
    wgT = nc.dram_tensor("wgT", [INNER, 2 * NH], BF16, kind="ExternalInput")   # [Wig.T | Wfg.T]
    convw = nc.dram_tensor("convw", [DH, (INNER // DH) * KCONV], F32, kind="ExternalInput")
    convb = nc.dram_tensor("convb", [DH, INNER // DH], F32, kind="ExternalInput")
    skip_i = nc.dram_tensor("skip_i", [DH, INNER // DH], F32, kind="ExternalInput")

    q_o = nc.dram_tensor("q_o", [INNER, TOK], BF16, kind="ExternalOutput")
    k_o = nc.dram_tensor("k_o", [INNER, TOK], BF16, kind="ExternalOutput")
    v_o = nc.dram_tensor("v_o", [INNER, TOK], BF16, kind="ExternalOutput")
    sxm_o = nc.dram_tensor("sxm_o", [INNER, TOK], BF16, kind="ExternalOutput")   # skip * xm
    g2_o = nc.dram_tensor("g2_o", [INNER, TOK], BF16, kind="ExternalOutput")     # silu(x_og)
    gates_o = nc.dram_tensor("gates_o", [2 * NH, TOK], F32, kind="ExternalOutput")

    KT_UP = D // DH          # 4 k-tiles for up-proj
    FT = INNER // DH         # 8 feature tiles of the mlstm half
    KT_IN = INNER // DH      # 8 k-tiles over INNER
    MT_QKV = 3 * FT          # 24
    HALO = KCONV - 1         # 3

    with tile.TileContext(nc) as tc, \
         tc.tile_pool(name="const", bufs=1) as const, \
         tc.tile_pool(name="big", bufs=1) as big, \
         tc.tile_pool(name="ev", bufs=3) as ev, \
         tc.tile_pool(name="wpool", bufs=4) as wpool, \
         tc.tile_pool(name="gev", bufs=1) as gev, \
         tc.tile_pool(name="cv", bufs=3) as cv, \
         tc.tile_pool(name="ps", bufs=2, space="PSUM") as ps:
        if True:
            # --- load weights / x interleaved in PE consumption order:
            # wup m-slices and xt token-chunks arrive just ahead of the
            # up-projection chains that need them.
            wup_sb = big.tile([DH, KT_UP, 2 * INNER], BF16)
            xt_sb = big.tile([DH, KT_UP, TH], BF16)
            XCH = [(0, 515), (515, 512), (1027, 512), (1539, 512)]
            # PE m-tile order for the up-proj (mlstm front-loaded, og fills)
            UP_ORDER = [0, 1, 2, 8, 3, 9, 4, 10, 5, 11, 6, 12, 7, 13, 14, 15]

            def wup_slice(j):
                nc.sync.dma_start(
                    out=wup_sb[:, :, j * DH:(j + 1) * DH],
                    in_=wupT[:, j * DH:(j + 1) * DH].rearrange("(kt p) m -> p kt m", p=DH))
            for j in UP_ORDER[:4]:
                wup_slice(j)
            for c, (c0, cn) in enumerate(XCH):
                for kt in range(KT_UP):
                    nc.sync.dma_start(out=xt_sb[:, kt, c0:c0 + cn],
                                      in_=xt[kt * DH:(kt + 1) * DH, c0:c0 + cn])
            for j in UP_ORDER[4:]:
                wup_slice(j)
            convw_sb = const.tile([DH, FT, KCONV], F32)
            nc.sync.dma_start(out=convw_sb, in_=convw[:].rearrange("p (ft t) -> p ft t", ft=FT))
            convb_sb = const.tile([DH, FT], F32)
            nc.sync.dma_start(out=convb_sb, in_=convb[:])
            skip_sb = const.tile([DH, FT], F32)
            nc.sync.dma_start(out=skip_sb, in_=skip_i[:])
            wg_sb = const.tile([DH, KT_IN, 2 * NH], BF16)
            nc.sync.dma_start(out=wg_sb, in_=wgT[:].rearrange("(kt p) m -> p kt m", p=DH))

            xpre_sb = big.tile([DH, FT, TH], BF16)   # [p, ft, halo+tok]
            xm_sb = big.tile([DH, FT, TOK], BF16)
            xog_sb = big.tile([DH, FT, TOK], BF16)

            # --- halo chains (tokens 0..2 of xpre), one psum tile + one evac
            pt_h = ps.tile([DH, FT, HALO], F32, tag="mm")
            for m in range(FT):
                for kt in range(KT_UP):
                    nc.tensor.matmul(
                        pt_h[:, m, :],
                        wup_sb[:, kt, m * DH:(m + 1) * DH],
                        xt_sb[:, kt, 0:HALO],
                        start=(kt == 0), stop=(kt == KT_UP - 1))
            nc.scalar.copy(xpre_sb[:, :, 0:HALO], pt_h[:])

            rot = [0]
            def evac(dst, src, eng=None):
                # PSUM sources: only ACT/DVE may read PSUM (not GPSIMD)
                if eng is None:
                    eng = 'ad'[rot[0] % 2]
                    rot[0] += 1
                if eng == 'a':
                    nc.scalar.copy(dst, src)
                else:
                    nc.vector.tensor_copy(dst, src)

            # conv per ft: taps 0/2 as DVE tensor_scalar (4x mode), tap 1 on
            # Pool / tap 3 on DVE scalar_tensor_tensor, final add on DVE,
            # sigmoid on ACT, silu-mult on Pool.
            def emit_conv(ft):
                def xs(tau):
                    return xpre_sb[:, ft, tau:tau + TOK]
                # taps on DVE tensor_scalar (4x mode); conv bias folded into
                # tap 0; adds split DVE/Pool; silu-mult on Pool (SBUF only)
                p0 = cv.tile([DH, TOK], BF16, tag="ca")
                nc.vector.tensor_scalar(p0[:], xs(0), convw_sb[:, ft, 0:1],
                                        convb_sb[:, ft:ft + 1], OP.mult, OP.add)
                p1 = cv.tile([DH, TOK], BF16, tag="cb")
                nc.vector.tensor_scalar_mul(p1[:], xs(1), convw_sb[:, ft, 1:2])
                p2 = cv.tile([DH, TOK], BF16, tag="ca")
                nc.vector.tensor_scalar_mul(p2[:], xs(2), convw_sb[:, ft, 2:3])
                p3 = cv.tile([DH, TOK], BF16, tag="cb")
                nc.vector.tensor_scalar_mul(p3[:], xs(3), convw_sb[:, ft, 3:4])
                q0 = cv.tile([DH, TOK], BF16, tag="ca")
                nc.gpsimd.tensor_tensor(q0[:], p0[:], p1[:], OP.add)
                q1 = cv.tile([DH, TOK], BF16, tag="cb")
                nc.vector.tensor_tensor(q1[:], p2[:], p3[:], OP.add)
                y = cv.tile([DH, TOK], BF16, tag="y", bufs=2)
                nc.vector.tensor_tensor(y[:], q0[:], q1[:], OP.add)
                sg = cv.tile([DH, TOK], BF16, tag="sg", bufs=2)
                nc.scalar.activation(sg[:], y[:], AF.Sigmoid)
                nc.gpsimd.tensor_tensor(xm_sb[:, ft, :], y[:], sg[:], OP.mult)
                # sxm = skip * xm (DVE 4x)
                sxm_t = ev.tile([DH, TOK], BF16, tag="out")
                nc.vector.tensor_scalar_mul(sxm_t[:], xm_sb[:, ft, :],
                                            skip_sb[:, ft:ft + 1])
                nc.sync.dma_start(out=sxm_o[ft * DH:(ft + 1) * DH, :], in_=sxm_t[:])

            # --- up-projection in UP_ORDER (j<8: mlstm m-tile j, then its
            # conv; j>=8: og m-tile j-8, single-copy evac)
            for j in UP_ORDER:
                pt = ps.tile([DH, 4, 512], F32, tag="mm")
                for ns in range(4):
                    for kt in range(KT_UP):
                        nc.tensor.matmul(
                            pt[:, ns, :],
                            wup_sb[:, kt, j * DH:(j + 1) * DH],
                            xt_sb[:, kt, HALO + ns * 512: HALO + (ns + 1) * 512],
                            start=(kt == 0), stop=(kt == KT_UP - 1))
                if j < FT:
                    evac(xpre_sb[:, j, HALO:HALO + TOK],
                         pt[:].rearrange("p a b -> p (a b)"), eng='a')
                    emit_conv(j)
                else:
                    evac(xog_sb[:, j - FT, :], pt[:].rearrange("p a b -> p (a b)"),
                         eng='a' if (j % 2) else 'd')

            # --- g2 = silu(x_og) from SBUF (runs during gates/qkv PE work)
            for m in range(FT):
                sg2 = cv.tile([DH, TOK], BF16, tag="sg", bufs=2)
                nc.scalar.activation(sg2[:], xog_sb[:, m, :], AF.Sigmoid)
                g2_t = ev.tile([DH, TOK], BF16, tag="out")
                if m % 2 == 0:
                    nc.gpsimd.tensor_tensor(g2_t[:], xog_sb[:, m, :], sg2[:], OP.mult)
                else:
                    nc.vector.tensor_tensor(g2_t[:], xog_sb[:, m, :], sg2[:], OP.mult)
                nc.sync.dma_start(out=g2_o[m * DH:(m + 1) * DH, :], in_=g2_t[:])

            # --- gates: [16, TOK] f32
            ptg = ps.tile([2 * NH, 4, 512], F32, tag="mm")
            for ns in range(4):
                for kt in range(KT_IN):
                    nc.tensor.matmul(
                        ptg[:, ns, :], wg_sb[:, kt, :],
                        xm_sb[:, kt, ns * 512:(ns + 1) * 512],
                        start=(kt == 0), stop=(kt == KT_IN - 1))
            for hf in range(2):
                gv = gev.tile([2 * NH, TOK // 2], F32, tag="gv")
                nc.vector.tensor_copy(
                    gv[:], ptg[:, hf * 2:(hf + 1) * 2, :].rearrange("p a b -> p (a b)"))
                nc.sync.dma_start(
                    out=gates_o[:, hf * (TOK // 2):(hf + 1) * (TOK // 2)], in_=gv[:])

            # --- q/k/v projections (streamed weights)
            qkv_outs = [q_o, k_o, v_o]
            for m in range(MT_QKV):
                # weights streamed on the ACT queue so they never wait
                # behind the big output DMAs on SP
                w_sb = wpool.tile([DH, KT_IN, DH], BF16, tag="w")
                nc.sync.dma_start(
                    out=w_sb,
                    in_=wqkvT[:, m * DH:(m + 1) * DH].rearrange("(kt p) m -> p kt m", p=DH))
                out_t = qkv_outs[m // FT]
                mf = m % FT
                pt = ps.tile([DH, 4, 512], F32, tag="mm")
                for ns in range(4):
                    for kt in range(KT_IN):
                        nc.tensor.matmul(
                            pt[:, ns, :], w_sb[:, kt, :],
                            xm_sb[:, kt, ns * 512:(ns + 1) * 512],
                            start=(kt == 0), stop=(kt == KT_IN - 1))
                ev_t = ev.tile([DH, TOK], BF16, tag="ev")
                evac(ev_t[:], pt[:].rearrange("p a b -> p (a b)"))
                nc.sync.dma_start(out=out_t[mf * DH:(mf + 1) * DH, :], in_=ev_t[:])
    nc.compile()
    return nc


# ---------------------------------------------------------------- phase B ----
def build_phase_b():
    nc = new_nc()
    ins = {}
    outs = {}
    for u in range(NUNIT):
        # feat-major q and (esc*cch)-scaled k
        ins[f"qT{u}"] = nc.dram_tensor(f"qT{u}", [DH, S], BF16, kind="ExternalInput")
        ins[f"kTc{u}"] = nc.dram_tensor(f"kTc{u}", [DH, S], BF16, kind="ExternalInput")
        # token-major (p = token-in-chunk): [p, c, d] esc-scaled k, [p, c, e] = [v | 1]
        ins[f"kesc{u}"] = nc.dram_tensor(f"kesc{u}", [DH, NCH * DH], BF16, kind="ExternalInput")
        ins[f"vone{u}"] = nc.dram_tensor(f"vone{u}", [DH, NCH * NW], BF16, kind="ExternalInput")
        # packed per-unit scalars: [p, {dec(col0=0), e2, e3}, NCH] f32
        ins[f"scal{u}"] = nc.dram_tensor(f"scal{u}", [DH, 3 * NCH], F32, kind="ExternalInput")
        # h out in [p, c, e] layout
        outs[f"h{u}"] = nc.dram_tensor(f"h{u}", [DH, NCH * DH], BF16, kind="ExternalOutput")
    mask_i = nc.dram_tensor("mask_i", [DH, 4 * DH], BF16, kind="ExternalInput")
    BDBG = bool(os.environ.get("BDBG"))
    if BDBG:
        for nm in ("dbgU0", "dbgCs0", "dbgdecf0", "dbgsp0", "dbgdraw0"):
            sz = NW * NCH if nm not in ("dbgsp0", "dbgdraw0") else 4 * DH * 4
            outs[nm] = nc.dram_tensor(nm, [DH, sz], BF16, kind="ExternalOutput")

    SB = 4          # chunks per block
    GRP = 2         # blocks per den/h-DMA group
    NE = NW * NCH   # 8256 elements per big ring buffer
    ESPLIT = 65     # scan e-split: DVE does e<65, Pool e>=65
    with tile.TileContext(nc) as tc, \
         tc.tile_pool(name="small", bufs=1) as small, \
         tc.tile_pool(name="sh", bufs=10) as sh, \
         tc.tile_pool(name="spb", bufs=3) as spb, \
         tc.tile_pool(name="hrb", bufs=4) as hrb, \
         tc.tile_pool(name="hgo", bufs=2) as hgo, \
         tc.tile_pool(name="den", bufs=2) as den, \
         tc.tile_pool(name="ps1", bufs=2, space="PSUM") as ps1, \
         tc.tile_pool(name="psn", bufs=3, space="PSUM") as psn:
        if True:
            def ring(name):
                return sh.tile([DH, NE], BF16, tag="sh", name=name)

            mask_sb = small.tile([DH, SB, DH], BF16, name="mask")
            nc.sync.dma_start(
                out=mask_sb, in_=mask_i[:].rearrange("p (b l) -> p b l", b=SB))
            T = {}
            for u in range(NUNIT):
                T[u] = dict(scal=small.tile([DH, 3, NCH], F32, name=f"scal{u}"))
                nc.sync.dma_start(
                    out=T[u]['scal'],
                    in_=ins[f"scal{u}"][:].rearrange("p (k c) -> p k c", k=3))
            # ring allocation order (11 bufs): kesc1 (12th) wraps onto kesc0,
            # which dies after stage_i(0).
            T[0]['kesc'] = ring("kesc0")
            T[0]['U'] = ring("U0")
            T[0]['vone'] = ring("vone0")
            T[0]['Cs'] = ring("Cs0")
            T[0]['qT'] = ring("qT0")
            T[0]['kTc'] = ring("kTc0")
            T[1]['vone'] = ring("vone1")
            T[1]['U'] = ring("U1")
            T[1]['Cs'] = ring("Cs1")
            T[1]['qT'] = ring("qT1")
            # wraps (11th/12th allocs -> slots 1/2): kesc1 -> kesc0 (dead
            # after stage_i(0)); kTc1 -> U0 (dead after scan(0))
            T[1]['kesc'] = ring("kesc1")
            T[1]['kTc'] = ring("kTc1")
            # decay tile shared by both units (rebuilt between scans)
            decf_sh = small.tile([DH, NE], BF16, name="decf_sh")
            T[0]['decf'] = decf_sh
            T[1]['decf'] = decf_sh

            HALF = NCH // 2
            def issue_unit_dmas(u):
                # all inputs on SP: u1's ring-slot waits only delay u1
                # issues, which are transfer-bound anyway
                eng = nc.sync
                for half in range(2):
                    ks = slice(half * HALF * DH, (half + 1) * HALF * DH)
                    vs = slice(half * HALF * NW, (half + 1) * HALF * NW)
                    eng.dma_start(out=T[u]['kesc'][:, ks],
                                  in_=ins[f"kesc{u}"][:, ks])
                    eng.dma_start(out=T[u]['vone'][:, vs],
                                  in_=ins[f"vone{u}"][:, vs])
                order = ('qT', 'kTc') if u == 0 else ('kTc', 'qT')
                for half in range(2):
                    ts = slice(half * (S // 2), (half + 1) * (S // 2))
                    for nm in order:
                        eng.dma_start(out=T[u][nm][:, ts],
                                      in_=ins[f"{nm}{u}"][:, ts])

            def kescv(u):
                return T[u]['kesc'][:, :NCH * DH].rearrange("p (c d) -> p c d", c=NCH)

            def vonev(u):
                return T[u]['vone'][:].rearrange("p (c e) -> p c e", c=NCH)

            def Uv(u):
                return T[u]['U'][:].rearrange("p (e c) -> p e c", e=NW)

            def Csv(u):
                return T[u]['Cs'][:].rearrange("p (e c) -> p e c", e=NW)

            def stage_i(u, half):
                for cb in range(half * (HALF // SB), (half + 1) * (HALF // SB)):
                    up = psn.tile([DH, SB, 256], F32, tag="nps")
                    for i in range(SB):
                        c = cb * SB + i
                        nc.tensor.matmul(
                            up[:, i, :NW], kescv(u)[:, c, :], vonev(u)[:, c, :],
                            start=True, stop=True)
                    # PSUM -> SBUF bf16, strided into [p, e, c] layout
                    dst = Uv(u)[:, :, cb * SB:(cb + 1) * SB].rearrange("p e c -> p c e")
                    nc.scalar.copy(dst, up[:, :, :NW])

            def build_decf(u):
                # decf[p, (e, c)] = dec[p, c]; col c=0 is 0 => resets the
                # carried state at each e-boundary during the scan.
                t = T[u]
                dec_bc = t['scal'][:, 0, :]
                c0 = ESPLIT * NCH
                nc.vector.tensor_copy(
                    t['decf'][:, :c0].rearrange("p (e c) -> p e c", e=ESPLIT),
                    dec_bc.unsqueeze(1).broadcast_to([DH, ESPLIT, NCH]))
                nc.gpsimd.tensor_copy(
                    t['decf'][:, c0:].rearrange("p (e c) -> p e c", e=NW - ESPLIT),
                    dec_bc.unsqueeze(1).broadcast_to([DH, NW - ESPLIT, NCH]))

            def scan(u):
                # scans are DVE-only on HW (TensorScalarPtr engine check)
                t = T[u]
                nc.vector.tensor_tensor_scan(
                    t['Cs'][:], t['decf'][:], t['U'][:],
                    0.0, OP.mult, OP.add)

            def stage_ii_group(u, g):
                t = T[u]
                c0 = g * GRP * SB                 # first chunk of group
                draw = den.tile([DH, GRP * SB], F32, tag="draw")
                hrs = []
                for blk in range(GRP):
                    cb = g * GRP + blk
                    sps = ps1.tile([DH, SB, DH], F32, tag="sps")
                    for i in range(SB):
                        c = cb * SB + i
                        csl = slice(c * LC, (c + 1) * LC)
                        nc.tensor.matmul(
                            sps[:, i, :], t['kTc'][:, csl], t['qT'][:, csl],
                            start=True, stop=True)
                    # Sp = S * mask (esc/cch scaling baked into kTc)
                    sp = spb.tile([DH, SB, DH], BF16, tag="sp")
                    nc.vector.tensor_tensor(sp[:], sps[:], mask_sb[:], OP.mult)
                    # num matmuls
                    if BDBG and u == 0 and g == 0 and blk == 0:
                        nc.sync.dma_start(out=outs["dbgsp0"][:, :SB * DH].rearrange("p (a b) -> p a b", a=SB), in_=sp[:])
                    nps = psn.tile([DH, SB, 256], F32, tag="nps")
                    for i in range(SB):
                        c = cb * SB + i
                        csl = slice(c * LC, (c + 1) * LC)
                        if c > 0:
                            nc.tensor.matmul(
                                nps[:, i, :NW], t['qT'][:, csl], Csv(u)[:, :, c - 1],
                                start=True, stop=False)
                        nc.tensor.matmul(
                            nps[:, i, :NW], sp[:, i, :], vonev(u)[:, c, :],
                            start=(c == 0), stop=True)
                    # raw den column (f32) + unscaled h (bf16) out of PSUM
                    nc.scalar.copy(
                        draw[:, blk * SB:(blk + 1) * SB], nps[:, :, DH])
                    hr = hrb.tile([DH, SB, DH], BF16, tag="hr")
                    if blk % 2 == 0:
                        nc.scalar.copy(hr[:], nps[:, :, :DH])
                    else:
                        nc.vector.tensor_copy(hr[:], nps[:, :, :DH])
                    hrs.append(hr)
                if BDBG and u == 0 and g == 0:
                    nc.gpsimd.dma_start(out=outs["dbgdraw0"][:, :GRP * SB],
                                      in_=draw[:])
                # batched den for the 16-chunk group: den = max(|raw|, e2) + e3
                gsl = slice(c0, c0 + GRP * SB)
                dabs = den.tile([DH, GRP * SB], F32, tag="dabs")
                nc.scalar.activation(dabs[:], draw[:], AF.Abs)
                dmx = den.tile([DH, GRP * SB], F32, tag="dmx")
                nc.vector.tensor_tensor(dmx[:], dabs[:], t['scal'][:, 1, gsl], OP.max)
                dpl = den.tile([DH, GRP * SB], F32, tag="dpl")
                nc.vector.tensor_tensor(dpl[:], dmx[:], t['scal'][:, 2, gsl], OP.add)
                rden = den.tile([DH, GRP * SB], F32, tag="rden")
                nc.vector.reciprocal(rden[:], dpl[:])
                # h = hr * rden (broadcast over d), batched out-DMA per group
                hg = hgo.tile([DH, GRP * SB, DH], BF16, tag="hg")
                for blk in range(GRP):
                    bsl = slice(blk * SB, (blk + 1) * SB)
                    rb = rden[:, bsl].unsqueeze(2).broadcast_to([DH, SB, DH])
                    nc.gpsimd.tensor_tensor(hg[:, bsl, :], hrs[blk][:], rb, OP.mult)
                nc.sync.dma_start(
                    out=outs[f"h{u}"][:, c0 * DH:(c0 + GRP * SB) * DH], in_=hg[:])

            issue_unit_dmas(0)
            issue_unit_dmas(1)
            with tc.high_priority():
                build_decf(0)
                stage_i(0, 0)
                stage_i(0, 1)
                scan(0)
            if BDBG:
                for nm, t_ in (("dbgU0", T[0]['U']), ("dbgCs0", T[0]['Cs']),
                               ("dbgdecf0", T[0]['decf'])):
                    nc.sync.dma_start(out=outs[nm][:], in_=t_[:])
            NG = NCH // (GRP * SB)
            u0_pre = list(range(NG // 2))           # first half of u0 groups
            for g in u0_pre:
                stage_ii_group(0, g)
            stage_i(1, 0)
            stage_ii_group(0, NG // 2)
            stage_i(1, 1)
            build_decf(1)
            scan(1)
            for g in range(NG // 2 + 1, NG):
                stage_ii_group(0, g)
            for g in range(NG):
                stage_ii_group(1, g)
    nc.compile()
    return nc


# ---------------------------------------------------------------- phase C ----
def build_phase_c():
    nc = new_nc()
    hg_i = nc.dram_tensor("hg_i", [INNER, TOK], BF16, kind="ExternalInput")  # (h+skip*xm)*silu(x_og)
    wdT = nc.dram_tensor("wdT", [INNER, D], BF16, kind="ExternalInput")
    out_o = nc.dram_tensor("out_o", [D, TOK], BF16, kind="ExternalOutput")

    FT = INNER // DH   # 8
    MT = D // DH       # 4
    NS = TOK // 512    # 4 token slices
    with tile.TileContext(nc) as tc, \
         tc.tile_pool(name="big", bufs=1) as big, \
         tc.tile_pool(name="ev", bufs=4) as ev, \
         tc.tile_pool(name="ps", bufs=4, space="PSUM") as ps:
        if True:
            wd_sb = big.tile([DH, FT, D], BF16)
            hg_sb = big.tile([DH, FT, TOK], BF16)
            # weight m-tile 0 first so the first chain starts early
            nc.sync.dma_start(out=wd_sb[:, :, :DH],
                              in_=wdT[:, :DH].rearrange("(ft p) m -> p ft m", p=DH))
            for ns in range(NS):
                tsl = slice(ns * 512, (ns + 1) * 512)
                nc.sync.dma_start(
                    out=hg_sb[:, :, tsl],
                    in_=hg_i[:, tsl].rearrange("(ft p) t -> p ft t", p=DH))
                if ns == 0:
                    nc.sync.dma_start(
                        out=wd_sb[:, :, DH:],
                        in_=wdT[:, DH:].rearrange("(ft p) m -> p ft m", p=DH))
            for ns in range(NS):
                tsl = slice(ns * 512, (ns + 1) * 512)
                for m in range(MT):
                    pt = ps.tile([DH, 512], F32)
                    for kt in range(FT):
                        nc.tensor.matmul(
                            pt[:], wd_sb[:, kt, m * DH:(m + 1) * DH],
                            hg_sb[:, kt, tsl],
                            start=(kt == 0), stop=(kt == FT - 1))
                    ot = ev.tile([DH, 512], BF16, tag="ot")
                    if m % 2 == 0:
                        nc.scalar.copy(ot[:], pt[:])
                    else:
                        nc.vector.tensor_copy(ot[:], pt[:])
                    nc.sync.dma_start(
                        out=out_o[m * DH:(m + 1) * DH, tsl], in_=ot[:])
    nc.compile()
    return nc


# ------------------------------------------------------------- host glue ----
def host_gate_math(i_pre, f_pre):
    """i_pre, f_pre: [B, NH, S] f32.  Returns dict of f32 arrays.

    Exports (per b, h):
      esc   [S]        exp(a_j - m_new(chunk))            per-token k scale
      cch   [NCH]      exp(m_new - ms - scaG)             per-chunk Sp scale
      dec   [NCH]      exp(scaG + ms - m_new), col0 = 0   scan multiplier
      e2,e3 [NCH, LC]  den clamp terms (qk_scale folded)
    """
    i_pre = i_pre.astype(np.float64)
    f_pre = f_pre.astype(np.float64)
    vecI = np.log(1.0 / (1.0 + np.exp(-i_pre)) + EPS)
    vecF = np.log(1.0 / (1.0 + np.exp(-f_pre)) + EPS)
    Ic = vecI.reshape(B, NH, NCH, LC)
    Fc = vecF.reshape(B, NH, NCH, LC)
    vecB = np.cumsum(Fc, axis=-1)
    scaG = vecB[..., -1]
    vecA = scaG[..., None] - vecB + Ic

    ms = np.zeros((B, NH, NCH))
    dec = np.zeros((B, NH, NCH))
    m_new_arr = np.zeros((B, NH, NCH))
    m = np.zeros((B, NH))
    for c in range(NCH):
        amax = vecA[:, :, c, :].max(-1)
        m_new = np.maximum(scaG[:, :, c] + m, amax)
        ms[:, :, c] = m
        dec[:, :, c] = np.exp(scaG[:, :, c] + m - m_new)
        m_new_arr[:, :, c] = m_new
        m = m_new
    escale = np.exp(vecA - m_new_arr[..., None])          # [B,NH,NCH,LC]
    cch = np.exp(m_new_arr - ms - scaG)                   # [B,NH,NCH]

    mask = np.tril(np.ones((LC, LC), bool))
    logD = vecB[..., :, None] - vecB[..., None, :] + Ic[..., None, :]
    logD = np.where(mask, logD, -np.inf)
    m_intra = logD.max(-1)
    m_comb = np.maximum(vecB + ms[..., None], m_intra)
    e2 = np.exp(-vecB - ms[..., None]) / QK_SCALE
    e3 = EPS * np.exp(m_comb - vecB - ms[..., None]) / QK_SCALE
    dec0 = dec.copy()
    dec0[:, :, 0] = 0.0
    return dict(
        esc=escale.reshape(B, NH, S).astype(np.float32),
        cch=cch.astype(np.float32),
        dec=dec0.astype(np.float32),
        e2=e2.astype(np.float32), e3=e3.astype(np.float32))


def prep_weights(W_up, Wq, Wk, Wv, W_ig, W_fg, conv_w, conv_b, skip, W_down):
    """Host-side weight packing (same for all cores)."""
    FT = INNER // DH
    wupT = _bf(W_up.T)                                         # [512, 2048]
    wqkvT = _bf(np.concatenate([Wq.T, Wk.T, Wv.T], axis=1))    # [1024, 3072]
    wgT = _bf(np.concatenate([W_ig.T, W_fg.T], axis=1))        # [1024, 16]
    convw = np.ascontiguousarray(
        conv_w.reshape(FT, DH, KCONV).transpose(1, 0, 2).reshape(DH, FT * KCONV)
    ).astype(np.float32)
    convb = np.ascontiguousarray(conv_b.reshape(FT, DH).T).astype(np.float32)
    skip_p = np.ascontiguousarray(skip.reshape(FT, DH).T).astype(np.float32)
    wdT = _bf(W_down.T)                                        # [1024, 512]
    mask4 = _bf(np.tile(np.tril(np.ones((DH, DH), np.float32)), (1, 4)))
    return dict(wupT=wupT, wqkvT=wqkvT, wgT=wgT, convw=convw, convb=convb,
                skip_p=skip_p, wdT=wdT, mask4=mask4)


def build_a_inmaps(x, wp):
    """Per-core phase A input maps.  Core c = (b=c//4, quarter=c%4)."""
    in_maps = []
    for c in range(8):
        b, qt = c // 4, c % 4
        s0 = qt * TOK
        xs = x[b, :, :].T                                       # [512, S] view
        if s0 == 0:
            xt = np.concatenate([np.zeros((D, KCONV - 1), np.float32),
                                 xs[:, :TOK]], axis=1)
        else:
            xt = xs[:, s0 - (KCONV - 1): s0 + TOK]
        in_maps.append(dict(
            xt=_bf(xt), wupT=wp['wupT'], wqkvT=wp['wqkvT'], wgT=wp['wgT'],
            convw=wp['convw'], convb=wp['convb'], skip_i=wp['skip_p']))
    return in_maps


def assemble_a_outputs(a_results, b_ig, b_fg):
    """Concatenate per-core phase A outputs into full feature-major tensors."""
    def cat(name):
        return np.stack([
            np.concatenate([a_results[b * 4 + qt][name] for qt in range(4)], axis=1)
            for b in range(B)])
    q_t, k_t, v_t = cat('q_o'), cat('k_o'), cat('v_o')          # [B, INNER, S] bf16
    sxm_t, g2_t = cat('sxm_o'), cat('g2_o')
    gates = cat('gates_o').astype(np.float32)                   # [B, 16, S]
    i_pre = gates[:, :NH, :] + np.asarray(b_ig, np.float32)[None, :, None]
    f_pre = gates[:, NH:, :] + np.asarray(b_fg, np.float32)[None, :, None]
    return q_t, k_t, v_t, sxm_t, g2_t, i_pre, f_pre


def build_b_inmaps(q_t, k_t, v_t, g, wp):
    """Per-core phase B inputs.  Core c handles units (b, 2h) where
    b = c // 4, heads (2*(c%4), 2*(c%4)+1)."""
    in_maps = []
    for c in range(8):
        b, hp = c // 4, c % 4
        m = {'mask_i': wp['mask4']}
        for u in range(NUNIT):
            h = 2 * hp + u
            rs = slice(h * DH, (h + 1) * DH)
            kf = k_t[b, rs, :].astype(np.float32)               # [128, S] feat-major
            esc = g['esc'][b, h]                                # [S]
            cch_tok = np.repeat(g['cch'][b, h], LC)             # [S]
            m[f"qT{u}"] = np.ascontiguousarray(q_t[b, rs, :])
            m[f"kTc{u}"] = _bf(kf * (esc * cch_tok)[None, :])
            # token-major [p, c, d] / [p, c, e]
            kesc = (kf.T * esc[:, None]).reshape(NCH, LC, DH)
            m[f"kesc{u}"] = _bf(kesc.transpose(1, 0, 2).reshape(DH, NCH * DH))
            vone = np.empty((NCH, LC, NW), np.float32)
            vone[:, :, :DH] = v_t[b, rs, :].astype(np.float32).T.reshape(NCH, LC, DH)
            vone[:, :, DH] = 1.0
            m[f"vone{u}"] = _bf(vone.transpose(1, 0, 2).reshape(DH, NCH * NW))
            scal = np.empty((DH, 3, NCH), np.float32)
            scal[:, 0, :] = g['dec'][b, h][None, :]
            scal[:, 1, :] = g['e2'][b, h].T
            scal[:, 2, :] = g['e3'][b, h].T
            m[f"scal{u}"] = np.ascontiguousarray(scal.reshape(DH, 3 * NCH))
        in_maps.append(m)
    return in_maps


def build_c_inmaps(b_results, sxm_t, g2_t, wp):
    """Assemble h from phase B [p, c, e] layouts into feature-major h_t,
    fold the gating (hg = (h + skip*xm) * silu(x_og)) during the
    reshard, then per-core phase C inputs."""
    h_t = np.empty((B, INNER, S), dtype=np.float32)
    for c in range(8):
        b, hp = c // 4, c % 4
        for u in range(NUNIT):
            h = 2 * hp + u
            # [p, c, e] -> [e, c*LC + p]
            harr = np.asarray(b_results[c][f"h{u}"]).astype(np.float32)
            harr = harr.reshape(DH, NCH, DH)
            h_t[b, h * DH:(h + 1) * DH, :] = (
                harr.transpose(2, 1, 0).reshape(DH, S))
    hg_t = _bf((h_t + np.asarray(sxm_t, np.float32))
               * np.asarray(g2_t, np.float32))
    in_maps = []
    for c in range(8):
        b, qt = c // 4, c % 4
        ts = slice(qt * TOK, (qt + 1) * TOK)
        in_maps.append(dict(
            hg_i=np.ascontiguousarray(hg_t[b, :, ts]),
            wdT=wp['wdT']))
    return in_maps


def assemble_output(c_results):
    out = np.empty((B, S, D), np.float32)
    for c in range(8):
        b, qt = c // 4, c % 4
        out[b, qt * TOK:(qt + 1) * TOK, :] = \
            np.asarray(c_results[c]['out_o']).astype(np.float32).T
    return out


# ------------------------------------------------------------------ entry ----
from concourse.bass_utils import run_bass_kernel_spmd as _run_spmd

_CACHE = {}


def _programs():
    if 'a' not in _CACHE:
        _CACHE['a'] = build_phase_a()
        _CACHE['b'] = build_phase_b()
        _CACHE['c'] = build_phase_c()
    return _CACHE['a'], _CACHE['b'], _CACHE['c']


def kernel(x, W_up, Wq, Wk, Wv, W_ig, b_ig, W_fg, b_fg, conv_w, conv_b, skip,
           W_down):
    x = np.asarray(x, np.float32)
    nc_a, nc_b, nc_c = _programs()
    cores = list(range(8))
    wp = prep_weights(W_up, Wq, Wk, Wv, W_ig, W_fg, conv_w, conv_b, skip, W_down)
    a_maps = build_a_inmaps(x, wp)
    ra = _run_spmd(nc_a, a_maps, core_ids=cores).results
    q_t, k_t, v_t, sxm_t, g2_t, i_pre, f_pre = assemble_a_outputs(ra, b_ig, b_fg)
    g = host_gate_math(i_pre, f_pre)
    b_maps = build_b_inmaps(q_t, k_t, v_t, g, wp)
    rb = _run_spmd(nc_b, b_maps, core_ids=cores).results
    c_maps = build_c_inmaps(rb, sxm_t, g2_t, wp)
    rc = _run_spmd(nc_c, c_maps, core_ids=cores).results
    return assemble_output(rc)


# revision 45
# speedup vs baseline: 1.0099x; 1.0021x over previous
"""Trainium2 Bass kernel for nn_ChunkwiseMLSTM (B=2, S=8192, D=512, INNER=1024, NH=8).

kernel(**inputs) -> np.ndarray [2, 8192, 512] f32.

Three SPMD launches on 8 NeuronCores:
  A: token-sharded projections (up-proj, causal conv+SiLU split across
     DVE/Pool with 4x tensor_scalar taps, q/k/v, gate pre-acts, skip*xm,
     silu(x_og) fused into single ACT ops straight from PSUM).
  B: head-sharded chunkwise mLSTM (LC=128 chunks):
     - stage i: inter-chunk state updates U_c = kesc_c^T @ [v|1] (PE),
       strided PSUM->SBUF evacs (ACT/Pool), then per-unit e-split
       tensor_tensor_scan on DVE+Pool with broadcast-AP decay input.
     - stage ii: per 4-chunk blocks: S matmuls (PE), mask-mult, num
       matmuls, f32 den column extraction; den clamp math batched per
       16-chunk group; h scaling + group-batched DMA out (Pool queue).
  C: token-sharded output gating + down-projection, 512-token-sliced
     streaming; host pre-folds h+skip*xm during the B->C reshard.
Host between launches: gate-derived scan scalars (O(B*NH*S)), weight
pre-transposition, resharding.
"""
import os
os.environ.setdefault("JAX_COMPILATION_CACHE_DIR",
                      os.path.expanduser("~/.cache/jax_bass_cache"))
os.environ.setdefault("JAX_PERSISTENT_CACHE_MIN_ENTRY_SIZE_BYTES", "0")
os.environ.setdefault("JAX_PERSISTENT_CACHE_MIN_COMPILE_TIME_SECS", "0")

import sys
if '/opt/trn_rl_repo' not in sys.path:
    sys.path.insert(0, '/opt/trn_rl_repo')

import numpy as np
import ml_dtypes

import concourse.bass as bass
import concourse.tile as tile
from concourse import mybir, bacc

F32 = mybir.dt.float32
BF16 = mybir.dt.bfloat16
AF = mybir.ActivationFunctionType
OP = mybir.AluOpType

B, S, D = 2, 8192, 512
INNER, NH, KCONV = 1024, 8, 4
DH = 128
EPS = 1e-6
LC = 128           # chunk length used on device (math is chunk-size invariant)
NCH = S // LC      # 64
NW = 129           # [C | n] state width
QK_SCALE = DH ** -0.5
TOK = S // 4       # tokens per core in phases A/C = 2048
TH = TOK + (KCONV - 1)   # 2051 with conv halo
NUNIT = 2          # (b,h) units per core in phase B


def _bf(x):
    return np.ascontiguousarray(np.asarray(x, np.float32).astype(ml_dtypes.bfloat16))


def new_nc():
    return bacc.Bacc(None, target_bir_lowering=False, debug=False)


# ---------------------------------------------------------------- phase A ----
def build_phase_a():
    nc = new_nc()
    xt = nc.dram_tensor("xt", [D, TH], BF16, kind="ExternalInput")            # x[b].T slice (halo)
    wupT = nc.dram_tensor("wupT", [D, 2 * INNER], BF16, kind="ExternalInput")  # W_up.T
    wqkvT = nc.dram_tensor("wqkvT", [INNER, 3 * INNER], BF16, kind="ExternalInput")
    wgT = nc.dram_tensor("wgT", [INNER, 2 * NH], BF16, kind="ExternalInput")   # [Wig.T | Wfg.T]
    convw = nc.dram_tensor("convw", [DH, (INNER // DH) * KCONV], F32, kind="ExternalInput")
    convb = nc.dram_tensor("convb", [DH, INNER // DH], F32, kind="ExternalInput")
    skip_i = nc.dram_tensor("skip_i", [DH, INNER // DH], F32, kind="ExternalInput")

    q_o = nc.dram_tensor("q_o", [INNER, TOK], BF16, kind="ExternalOutput")
    k_o = nc.dram_tensor("k_o", [INNER, TOK], BF16, kind="ExternalOutput")
    v_o = nc.dram_tensor("v_o", [INNER, TOK], BF16, kind="ExternalOutput")
    sxm_o = nc.dram_tensor("sxm_o", [INNER, TOK], BF16, kind="ExternalOutput")   # skip * xm
    g2_o = nc.dram_tensor("g2_o", [INNER, TOK], BF16, kind="ExternalOutput")     # silu(x_og)
    gates_o = nc.dram_tensor("gates_o", [2 * NH, TOK], F32, kind="ExternalOutput")

    KT_UP = D // DH          # 4 k-tiles for up-proj
    FT = INNER // DH         # 8 feature tiles of the mlstm half
    KT_IN = INNER // DH      # 8 k-tiles over INNER
    MT_QKV = 3 * FT          # 24
    HALO = KCONV - 1         # 3

    with tile.TileContext(nc) as tc, \
         tc.tile_pool(name="const", bufs=1) as const, \
         tc.tile_pool(name="big", bufs=1) as big, \
         tc.tile_pool(name="ev", bufs=3) as ev, \
         tc.tile_pool(name="wpool", bufs=4) as wpool, \
         tc.tile_pool(name="gev", bufs=1) as gev, \
         tc.tile_pool(name="cv", bufs=3) as cv, \
         tc.tile_pool(name="ps", bufs=2, space="PSUM") as ps:
        if True:
            # --- load weights / x interleaved in PE consumption order:
            # wup m-slices and xt token-chunks arrive just ahead of the
            # up-projection chains that need them.
            wup_sb = big.tile([DH, KT_UP, 2 * INNER], BF16)
            xt_sb = big.tile([DH, KT_UP, TH], BF16)
            XCH = [(0, 515), (515, 512), (1027, 512), (1539, 512)]
            # PE m-tile order for the up-proj (mlstm front-loaded, og fills)
            UP_ORDER = [0, 1, 2, 8, 3, 9, 4, 10, 5, 11, 6, 12, 7, 13, 14, 15]

            def wup_slice(j):
                nc.sync.dma_start(
                    out=wup_sb[:, :, j * DH:(j + 1) * DH],
                    in_=wupT[:, j * DH:(j + 1) * DH].rearrange("(kt p) m -> p kt m", p=DH))
            for j in UP_ORDER[:4]:
                wup_slice(j)
            for c, (c0, cn) in enumerate(XCH):
                for kt in range(KT_UP):
                    nc.sync.dma_start(out=xt_sb[:, kt, c0:c0 + cn],
                                      in_=xt[kt * DH:(kt + 1) * DH, c0:c0 + cn])
            for j in UP_ORDER[4:]:
                wup_slice(j)
            convw_sb = const.tile([DH, FT, KCONV], F32)
            nc.sync.dma_start(out=convw_sb, in_=convw[:].rearrange("p (ft t) -> p ft t", ft=FT))
            convb_sb = const.tile([DH, FT], F32)
            nc.sync.dma_start(out=convb_sb, in_=convb[:])
            skip_sb = const.tile([DH, FT], F32)
            nc.sync.dma_start(out=skip_sb, in_=skip_i[:])
            wg_sb = const.tile([DH, KT_IN, 2 * NH], BF16)
            nc.sync.dma_start(out=wg_sb, in_=wgT[:].rearrange("(kt p) m -> p kt m", p=DH))

            xpre_sb = big.tile([DH, FT, TH], BF16)   # [p, ft, halo+tok]
            xm_sb = big.tile([DH, FT, TOK], BF16)
            xog_sb = big.tile([DH, FT, TOK], BF16)

            # --- halo chains (tokens 0..2 of xpre), one psum tile + one evac
            pt_h = ps.tile([DH, FT, HALO], F32, tag="mm")
            for m in range(FT):
                for kt in range(KT_UP):
                    nc.tensor.matmul(
                        pt_h[:, m, :],
                        wup_sb[:, kt, m * DH:(m + 1) * DH],
                        xt_sb[:, kt, 0:HALO],
                        start=(kt == 0), stop=(kt == KT_UP - 1))
            nc.scalar.copy(xpre_sb[:, :, 0:HALO], pt_h[:])

            rot = [0]
            def evac(dst, src, eng=None):
                # PSUM sources: only ACT/DVE may read PSUM (not GPSIMD)
                if eng is None:
                    eng = 'ad'[rot[0] % 2]
                    rot[0] += 1
                if eng == 'a':
                    nc.scalar.copy(dst, src)
                else:
                    nc.vector.tensor_copy(dst, src)

            # conv per ft: taps 0/2 as DVE tensor_scalar (4x mode), tap 1 on
            # Pool / tap 3 on DVE scalar_tensor_tensor, final add on DVE,
            # sigmoid on ACT, silu-mult on Pool.
            def emit_conv(ft):
                def xs(tau):
                    return xpre_sb[:, ft, tau:tau + TOK]
                # taps on DVE tensor_scalar (4x mode); conv bias folded into
                # tap 0; adds split DVE/Pool; silu-mult on Pool (SBUF only)
                p0 = cv.tile([DH, TOK], BF16, tag="ca")
                nc.vector.tensor_scalar(p0[:], xs(0), convw_sb[:, ft, 0:1],
                                        convb_sb[:, ft:ft + 1], OP.mult, OP.add)
                p1 = cv.tile([DH, TOK], BF16, tag="cb")
                nc.vector.tensor_scalar_mul(p1[:], xs(1), convw_sb[:, ft, 1:2])
                p2 = cv.tile([DH, TOK], BF16, tag="ca")
                nc.vector.tensor_scalar_mul(p2[:], xs(2), convw_sb[:, ft, 2:3])
                p3 = cv.tile([DH, TOK], BF16, tag="cb")
                nc.vector.tensor_scalar_mul(p3[:], xs(3), convw_sb[:, ft, 3:4])
                q0 = cv.tile([DH, TOK], BF16, tag="ca")
                nc.gpsimd.tensor_tensor(q0[:], p0[:], p1[:], OP.add)
                q1 = cv.tile([DH, TOK], BF16, tag="cb")
                nc.vector.tensor_tensor(q1[:], p2[:], p3[:], OP.add)
                y = cv.tile([DH, TOK], BF16, tag="y", bufs=2)
                nc.vector.tensor_tensor(y[:], q0[:], q1[:], OP.add)
                sg = cv.tile([DH, TOK], BF16, tag="sg", bufs=2)
                nc.scalar.activation(sg[:], y[:], AF.Sigmoid)
                nc.gpsimd.tensor_tensor(xm_sb[:, ft, :], y[:], sg[:], OP.mult)
                # sxm = skip * xm (DVE 4x)
                sxm_t = ev.tile([DH, TOK], BF16, tag="out")
                nc.vector.tensor_scalar_mul(sxm_t[:], xm_sb[:, ft, :],
                                            skip_sb[:, ft:ft + 1])
                nc.sync.dma_start(out=sxm_o[ft * DH:(ft + 1) * DH, :], in_=sxm_t[:])

            # --- up-projection in UP_ORDER (j<8: mlstm m-tile j, then its
            # conv; j>=8: og m-tile j-8, single-copy evac)
            for j in UP_ORDER:
                pt = ps.tile([DH, 4, 512], F32, tag="mm")
                for ns in range(4):
                    for kt in range(KT_UP):
                        nc.tensor.matmul(
                            pt[:, ns, :],
                            wup_sb[:, kt, j * DH:(j + 1) * DH],
                            xt_sb[:, kt, HALO + ns * 512: HALO + (ns + 1) * 512],
                            start=(kt == 0), stop=(kt == KT_UP - 1))
                if j < FT:
                    evac(xpre_sb[:, j, HALO:HALO + TOK],
                         pt[:].rearrange("p a b -> p (a b)"), eng='a')
                    emit_conv(j)
                else:
                    evac(xog_sb[:, j - FT, :], pt[:].rearrange("p a b -> p (a b)"),
                         eng='a' if (j % 2) else 'd')

            # --- g2 = silu(x_og) from SBUF (runs during gates/qkv PE work)
            for m in range(FT):
                sg2 = cv.tile([DH, TOK], BF16, tag="sg", bufs=2)
                nc.scalar.activation(sg2[:], xog_sb[:, m, :], AF.Sigmoid)
                g2_t = ev.tile([DH, TOK], BF16, tag="out")
                if m % 2 == 0:
                    nc.gpsimd.tensor_tensor(g2_t[:], xog_sb[:, m, :], sg2[:], OP.mult)
                else:
                    nc.vector.tensor_tensor(g2_t[:], xog_sb[:, m, :], sg2[:], OP.mult)
                nc.sync.dma_start(out=g2_o[m * DH:(m + 1) * DH, :], in_=g2_t[:])

            # --- gates: [16, TOK] f32
            ptg = ps.tile([2 * NH, 4, 512], F32, tag="mm")
            for ns in range(4):
                for kt in range(KT_IN):
                    nc.tensor.matmul(
                        ptg[:, ns, :], wg_sb[:, kt, :],
                        xm_sb[:, kt, ns * 512:(ns + 1) * 512],
                        start=(kt == 0), stop=(kt == KT_IN - 1))
            for hf in range(2):
                gv = gev.tile([2 * NH, TOK // 2], F32, tag="gv")
                nc.vector.tensor_copy(
                    gv[:], ptg[:, hf * 2:(hf + 1) * 2, :].rearrange("p a b -> p (a b)"))
                nc.sync.dma_start(
                    out=gates_o[:, hf * (TOK // 2):(hf + 1) * (TOK // 2)], in_=gv[:])

            # --- q/k/v projections (streamed weights)
            qkv_outs = [q_o, k_o, v_o]
            for m in range(MT_QKV):
                # weights streamed on the ACT queue so they never wait
                # behind the big output DMAs on SP
                w_sb = wpool.tile([DH, KT_IN, DH], BF16, tag="w")
                nc.sync.dma_start(
                    out=w_sb,
                    in_=wqkvT[:, m * DH:(m + 1) * DH].rearrange("(kt p) m -> p kt m", p=DH))
                out_t = qkv_outs[m // FT]
                mf = m % FT
                pt = ps.tile([DH, 4, 512], F32, tag="mm")
                for ns in range(4):
                    for kt in range(KT_IN):
                        nc.tensor.matmul(
                            pt[:, ns, :], w_sb[:, kt, :],
                            xm_sb[:, kt, ns * 512:(ns + 1) * 512],
                            start=(kt == 0), stop=(kt == KT_IN - 1))
                ev_t = ev.tile([DH, TOK], BF16, tag="ev")
                evac(ev_t[:], pt[:].rearrange("p a b -> p (a b)"))
                nc.sync.dma_start(out=out_t[mf * DH:(mf + 1) * DH, :], in_=ev_t[:])
    nc.compile()
    return nc


# ---------------------------------------------------------------- phase B ----
def build_phase_b():
    nc = new_nc()
    ins = {}
    outs = {}
    for u in range(NUNIT):
        # feat-major q and (esc*cch)-scaled k
        ins[f"qT{u}"] = nc.dram_tensor(f"qT{u}", [DH, S], BF16, kind="ExternalInput")
        ins[f"kTc{u}"] = nc.dram_tensor(f"kTc{u}", [DH, S], BF16, kind="ExternalInput")
        # token-major (p = token-in-chunk): [p, c, d] esc-scaled k, [p, c, e] = [v | 1]
        ins[f"kesc{u}"] = nc.dram_tensor(f"kesc{u}", [DH, NCH * DH], BF16, kind="ExternalInput")
        ins[f"vone{u}"] = nc.dram_tensor(f"vone{u}", [DH, NCH * NW], BF16, kind="ExternalInput")
        # packed per-unit scalars: [p, {dec(col0=0), e2, e3}, NCH] f32
        ins[f"scal{u}"] = nc.dram_tensor(f"scal{u}", [DH, 3 * NCH], F32, kind="ExternalInput")
        # h out in [p, c, e] layout
        outs[f"h{u}"] = nc.dram_tensor(f"h{u}", [DH, NCH * DH], BF16, kind="ExternalOutput")
    mask_i = nc.dram_tensor("mask_i", [DH, 4 * DH], BF16, kind="ExternalInput")
    BDBG = bool(os.environ.get("BDBG"))
    if BDBG:
        for nm in ("dbgU0", "dbgCs0", "dbgdecf0", "dbgsp0", "dbgdraw0"):
            sz = NW * NCH if nm not in ("dbgsp0", "dbgdraw0") else 4 * DH * 4
            outs[nm] = nc.dram_tensor(nm, [DH, sz], BF16, kind="ExternalOutput")

    SB = 4          # chunks per block
    GRP = 2         # blocks per den/h-DMA group
    NE = NW * NCH   # 8256 elements per big ring buffer
    ESPLIT = 65     # scan e-split: DVE does e<65, Pool e>=65
    with tile.TileContext(nc) as tc, \
         tc.tile_pool(name="small", bufs=1) as small, \
         tc.tile_pool(name="sh", bufs=10) as sh, \
         tc.tile_pool(name="spb", bufs=4) as spb, \
         tc.tile_pool(name="hrb", bufs=6) as hrb, \
         tc.tile_pool(name="hgo", bufs=4) as hgo, \
         tc.tile_pool(name="den", bufs=2) as den, \
         tc.tile_pool(name="ps1", bufs=2, space="PSUM") as ps1, \
         tc.tile_pool(name="psn", bufs=3, space="PSUM") as psn:
        if True:
            def ring(name):
                return sh.tile([DH, NE], BF16, tag="sh", name=name)

            mask_sb = small.tile([DH, SB, DH], BF16, name="mask")
            nc.sync.dma_start(
                out=mask_sb, in_=mask_i[:].rearrange("p (b l) -> p b l", b=SB))
            T = {}
            for u in range(NUNIT):
                T[u] = dict(scal=small.tile([DH, 3, NCH], F32, name=f"scal{u}"))
                nc.sync.dma_start(
                    out=T[u]['scal'],
                    in_=ins[f"scal{u}"][:].rearrange("p (k c) -> p k c", k=3))
            # ring allocation order (11 bufs): kesc1 (12th) wraps onto kesc0,
            # which dies after stage_i(0).
            T[0]['kesc'] = ring("kesc0")
            T[0]['U'] = ring("U0")
            T[0]['vone'] = ring("vone0")
            T[0]['Cs'] = ring("Cs0")
            T[0]['qT'] = ring("qT0")
            T[0]['kTc'] = ring("kTc0")
            T[1]['vone'] = ring("vone1")
            T[1]['U'] = ring("U1")
            T[1]['Cs'] = ring("Cs1")
            T[1]['qT'] = ring("qT1")
            # wraps (11th/12th allocs -> slots 1/2): kesc1 -> kesc0 (dead
            # after stage_i(0)); kTc1 -> U0 (dead after scan(0))
            T[1]['kesc'] = ring("kesc1")
            T[1]['kTc'] = ring("kTc1")
            # decay tile shared by both units (rebuilt between scans)
            decf_sh = small.tile([DH, NE], BF16, name="decf_sh")
            T[0]['decf'] = decf_sh
            T[1]['decf'] = decf_sh

            HALF = NCH // 2
            def issue_unit_dmas(u):
                # all inputs on SP: u1's ring-slot waits only delay u1
                # issues, which are transfer-bound anyway
                eng = nc.sync
                for half in range(2):
                    ks = slice(half * HALF * DH, (half + 1) * HALF * DH)
                    vs = slice(half * HALF * NW, (half + 1) * HALF * NW)
                    eng.dma_start(out=T[u]['kesc'][:, ks],
                                  in_=ins[f"kesc{u}"][:, ks])
                    eng.dma_start(out=T[u]['vone'][:, vs],
                                  in_=ins[f"vone{u}"][:, vs])
                order = ('qT', 'kTc') if u == 0 else ('kTc', 'qT')
                for half in range(2):
                    ts = slice(half * (S // 2), (half + 1) * (S // 2))
                    for nm in order:
                        eng.dma_start(out=T[u][nm][:, ts],
                                      in_=ins[f"{nm}{u}"][:, ts])

            def kescv(u):
                return T[u]['kesc'][:, :NCH * DH].rearrange("p (c d) -> p c d", c=NCH)

            def vonev(u):
                return T[u]['vone'][:].rearrange("p (c e) -> p c e", c=NCH)

            def Uv(u):
                return T[u]['U'][:].rearrange("p (e c) -> p e c", e=NW)

            def Csv(u):
                return T[u]['Cs'][:].rearrange("p (e c) -> p e c", e=NW)

            def stage_i(u, half):
                for cb in range(half * (HALF // SB), (half + 1) * (HALF // SB)):
                    up = psn.tile([DH, SB, 256], F32, tag="nps")
                    for i in range(SB):
                        c = cb * SB + i
                        nc.tensor.matmul(
                            up[:, i, :NW], kescv(u)[:, c, :], vonev(u)[:, c, :],
                            start=True, stop=True)
                    # PSUM -> SBUF bf16, strided into [p, e, c] layout
                    dst = Uv(u)[:, :, cb * SB:(cb + 1) * SB].rearrange("p e c -> p c e")
                    nc.scalar.copy(dst, up[:, :, :NW])

            def build_decf(u):
                # decf[p, (e, c)] = dec[p, c]; col c=0 is 0 => resets the
                # carried state at each e-boundary during the scan.
                t = T[u]
                dec_bc = t['scal'][:, 0, :]
                c0 = ESPLIT * NCH
                nc.vector.tensor_copy(
                    t['decf'][:, :c0].rearrange("p (e c) -> p e c", e=ESPLIT),
                    dec_bc.unsqueeze(1).broadcast_to([DH, ESPLIT, NCH]))
                nc.gpsimd.tensor_copy(
                    t['decf'][:, c0:].rearrange("p (e c) -> p e c", e=NW - ESPLIT),
                    dec_bc.unsqueeze(1).broadcast_to([DH, NW - ESPLIT, NCH]))

            def scan(u):
                # scans are DVE-only on HW (TensorScalarPtr engine check)
                t = T[u]
                nc.vector.tensor_tensor_scan(
                    t['Cs'][:], t['decf'][:], t['U'][:],
                    0.0, OP.mult, OP.add)

            def stage_ii_group(u, g):
                t = T[u]
                c0 = g * GRP * SB                 # first chunk of group
                draw = den.tile([DH, GRP * SB], F32, tag="draw")
                hrs = []
                for blk in range(GRP):
                    cb = g * GRP + blk
                    sps = ps1.tile([DH, SB, DH], F32, tag="sps")
                    for i in range(SB):
                        c = cb * SB + i
                        csl = slice(c * LC, (c + 1) * LC)
                        nc.tensor.matmul(
                            sps[:, i, :], t['kTc'][:, csl], t['qT'][:, csl],
                            start=True, stop=True)
                    # Sp = S * mask (esc/cch scaling baked into kTc)
                    sp = spb.tile([DH, SB, DH], BF16, tag="sp")
                    nc.vector.tensor_tensor(sp[:], sps[:], mask_sb[:], OP.mult)
                    # num matmuls
                    if BDBG and u == 0 and g == 0 and blk == 0:
                        nc.sync.dma_start(out=outs["dbgsp0"][:, :SB * DH].rearrange("p (a b) -> p a b", a=SB), in_=sp[:])
                    nps = psn.tile([DH, SB, 256], F32, tag="nps")
                    for i in range(SB):
                        c = cb * SB + i
                        csl = slice(c * LC, (c + 1) * LC)
                        if c > 0:
                            nc.tensor.matmul(
                                nps[:, i, :NW], t['qT'][:, csl], Csv(u)[:, :, c - 1],
                                start=True, stop=False)
                        nc.tensor.matmul(
                            nps[:, i, :NW], sp[:, i, :], vonev(u)[:, c, :],
                            start=(c == 0), stop=True)
                    # raw den column (f32) + unscaled h (bf16) out of PSUM
                    nc.scalar.copy(
                        draw[:, blk * SB:(blk + 1) * SB], nps[:, :, DH])
                    hr = hrb.tile([DH, SB, DH], BF16, tag="hr")
                    if blk % 2 == 0:
                        nc.scalar.copy(hr[:], nps[:, :, :DH])
                    else:
                        nc.vector.tensor_copy(hr[:], nps[:, :, :DH])
                    hrs.append(hr)
                if BDBG and u == 0 and g == 0:
                    nc.gpsimd.dma_start(out=outs["dbgdraw0"][:, :GRP * SB],
                                      in_=draw[:])
                # batched den for the 16-chunk group: den = max(|raw|, e2) + e3
                gsl = slice(c0, c0 + GRP * SB)
                dabs = den.tile([DH, GRP * SB], F32, tag="dabs")
                nc.scalar.activation(dabs[:], draw[:], AF.Abs)
                dmx = den.tile([DH, GRP * SB], F32, tag="dmx")
                nc.vector.tensor_tensor(dmx[:], dabs[:], t['scal'][:, 1, gsl], OP.max)
                dpl = den.tile([DH, GRP * SB], F32, tag="dpl")
                nc.vector.tensor_tensor(dpl[:], dmx[:], t['scal'][:, 2, gsl], OP.add)
                rden = den.tile([DH, GRP * SB], F32, tag="rden")
                nc.vector.reciprocal(rden[:], dpl[:])
                # h = hr * rden (broadcast over d), batched out-DMA per group
                hg = hgo.tile([DH, GRP * SB, DH], BF16, tag="hg")
                for blk in range(GRP):
                    bsl = slice(blk * SB, (blk + 1) * SB)
                    rb = rden[:, bsl].unsqueeze(2).broadcast_to([DH, SB, DH])
                    nc.gpsimd.tensor_tensor(hg[:, bsl, :], hrs[blk][:], rb, OP.mult)
                nc.sync.dma_start(
                    out=outs[f"h{u}"][:, c0 * DH:(c0 + GRP * SB) * DH], in_=hg[:])

            issue_unit_dmas(0)
            issue_unit_dmas(1)
            with tc.high_priority():
                build_decf(0)
                stage_i(0, 0)
                stage_i(0, 1)
                scan(0)
            if BDBG:
                for nm, t_ in (("dbgU0", T[0]['U']), ("dbgCs0", T[0]['Cs']),
                               ("dbgdecf0", T[0]['decf'])):
                    nc.sync.dma_start(out=outs[nm][:], in_=t_[:])
            NG = NCH // (GRP * SB)
            u0_pre = list(range(NG // 2))           # first half of u0 groups
            for g in u0_pre:
                stage_ii_group(0, g)
            stage_i(1, 0)
            stage_ii_group(0, NG // 2)
            stage_i(1, 1)
            build_decf(1)
            scan(1)
            for g in range(NG // 2 + 1, NG):
                stage_ii_group(0, g)
            for g in range(NG):
                stage_ii_group(1, g)
    nc.compile()
    return nc


# ---------------------------------------------------------------- phase C ----
def build_phase_c():
    nc = new_nc()
    hg_i = nc.dram_tensor("hg_i", [INNER, TOK], BF16, kind="ExternalInput")  # (h+skip*xm)*silu(x_og)
    wdT = nc.dram_tensor("wdT", [INNER, D], BF16, kind="ExternalInput")
    out_o = nc.dram_tensor("out_o", [D, TOK], BF16, kind="ExternalOutput")

    FT = INNER // DH   # 8
    MT = D // DH       # 4
    NS = TOK // 512    # 4 token slices
    with tile.TileContext(nc) as tc, \
         tc.tile_pool(name="big", bufs=1) as big, \
         tc.tile_pool(name="ev", bufs=4) as ev, \
         tc.tile_pool(name="ps", bufs=4, space="PSUM") as ps:
        if True:
            wd_sb = big.tile([DH, FT, D], BF16)
            hg_sb = big.tile([DH, FT, TOK], BF16)
            # weight m-tile 0 first so the first chain starts early
            nc.sync.dma_start(out=wd_sb[:, :, :DH],
                              in_=wdT[:, :DH].rearrange("(ft p) m -> p ft m", p=DH))
            for ns in range(NS):
                tsl = slice(ns * 512, (ns + 1) * 512)
                nc.sync.dma_start(
                    out=hg_sb[:, :, tsl],
                    in_=hg_i[:, tsl].rearrange("(ft p) t -> p ft t", p=DH))
                if ns == 0:
                    nc.sync.dma_start(
                        out=wd_sb[:, :, DH:],
                        in_=wdT[:, DH:].rearrange("(ft p) m -> p ft m", p=DH))
            for ns in range(NS):
                tsl = slice(ns * 512, (ns + 1) * 512)
                for m in range(MT):
                    pt = ps.tile([DH, 512], F32)
                    for kt in range(FT):
                        nc.tensor.matmul(
                            pt[:], wd_sb[:, kt, m * DH:(m + 1) * DH],
                            hg_sb[:, kt, tsl],
                            start=(kt == 0), stop=(kt == FT - 1))
                    ot = ev.tile([DH, 512], BF16, tag="ot")
                    if m % 2 == 0:
                        nc.scalar.copy(ot[:], pt[:])
                    else:
                        nc.vector.tensor_copy(ot[:], pt[:])
                    nc.sync.dma_start(
                        out=out_o[m * DH:(m + 1) * DH, tsl], in_=ot[:])
    nc.compile()
    return nc


# ------------------------------------------------------------- host glue ----
def host_gate_math(i_pre, f_pre):
    """i_pre, f_pre: [B, NH, S] f32.  Returns dict of f32 arrays.

    Exports (per b, h):
      esc   [S]        exp(a_j - m_new(chunk))            per-token k scale
      cch   [NCH]      exp(m_new - ms - scaG)             per-chunk Sp scale
      dec   [NCH]      exp(scaG + ms - m_new), col0 = 0   scan multiplier
      e2,e3 [NCH, LC]  den clamp terms (qk_scale folded)
    """
    i_pre = i_pre.astype(np.float64)
    f_pre = f_pre.astype(np.float64)
    vecI = np.log(1.0 / (1.0 + np.exp(-i_pre)) + EPS)
    vecF = np.log(1.0 / (1.0 + np.exp(-f_pre)) + EPS)
    Ic = vecI.reshape(B, NH, NCH, LC)
    Fc = vecF.reshape(B, NH, NCH, LC)
    vecB = np.cumsum(Fc, axis=-1)
    scaG = vecB[..., -1]
    vecA = scaG[..., None] - vecB + Ic

    ms = np.zeros((B, NH, NCH))
    dec = np.zeros((B, NH, NCH))
    m_new_arr = np.zeros((B, NH, NCH))
    m = np.zeros((B, NH))
    for c in range(NCH):
        amax = vecA[:, :, c, :].max(-1)
        m_new = np.maximum(scaG[:, :, c] + m, amax)
        ms[:, :, c] = m
        dec[:, :, c] = np.exp(scaG[:, :, c] + m - m_new)
        m_new_arr[:, :, c] = m_new
        m = m_new
    escale = np.exp(vecA - m_new_arr[..., None])          # [B,NH,NCH,LC]
    cch = np.exp(m_new_arr - ms - scaG)                   # [B,NH,NCH]

    mask = np.tril(np.ones((LC, LC), bool))
    logD = vecB[..., :, None] - vecB[..., None, :] + Ic[..., None, :]
    logD = np.where(mask, logD, -np.inf)
    m_intra = logD.max(-1)
    m_comb = np.maximum(vecB + ms[..., None], m_intra)
    e2 = np.exp(-vecB - ms[..., None]) / QK_SCALE
    e3 = EPS * np.exp(m_comb - vecB - ms[..., None]) / QK_SCALE
    dec0 = dec.copy()
    dec0[:, :, 0] = 0.0
    return dict(
        esc=escale.reshape(B, NH, S).astype(np.float32),
        cch=cch.astype(np.float32),
        dec=dec0.astype(np.float32),
        e2=e2.astype(np.float32), e3=e3.astype(np.float32))


def prep_weights(W_up, Wq, Wk, Wv, W_ig, W_fg, conv_w, conv_b, skip, W_down):
    """Host-side weight packing (same for all cores)."""
    FT = INNER // DH
    wupT = _bf(W_up.T)                                         # [512, 2048]
    wqkvT = _bf(np.concatenate([Wq.T, Wk.T, Wv.T], axis=1))    # [1024, 3072]
    wgT = _bf(np.concatenate([W_ig.T, W_fg.T], axis=1))        # [1024, 16]
    convw = np.ascontiguousarray(
        conv_w.reshape(FT, DH, KCONV).transpose(1, 0, 2).reshape(DH, FT * KCONV)
    ).astype(np.float32)
    convb = np.ascontiguousarray(conv_b.reshape(FT, DH).T).astype(np.float32)
    skip_p = np.ascontiguousarray(skip.reshape(FT, DH).T).astype(np.float32)
    wdT = _bf(W_down.T)                                        # [1024, 512]
    mask4 = _bf(np.tile(np.tril(np.ones((DH, DH), np.float32)), (1, 4)))
    return dict(wupT=wupT, wqkvT=wqkvT, wgT=wgT, convw=convw, convb=convb,
                skip_p=skip_p, wdT=wdT, mask4=mask4)


def build_a_inmaps(x, wp):
    """Per-core phase A input maps.  Core c = (b=c//4, quarter=c%4)."""
    in_maps = []
    for c in range(8):
        b, qt = c // 4, c % 4
        s0 = qt * TOK
        xs = x[b, :, :].T                                       # [512, S] view
        if s0 == 0:
            xt = np.concatenate([np.zeros((D, KCONV - 1), np.float32),
                                 xs[:, :TOK]], axis=1)
        else:
            xt = xs[:, s0 - (KCONV - 1): s0 + TOK]
        in_maps.append(dict(
            xt=_bf(xt), wupT=wp['wupT'], wqkvT=wp['wqkvT'], wgT=wp['wgT'],
            convw=wp['convw'], convb=wp['convb'], skip_i=wp['skip_p']))
    return in_maps


def assemble_a_outputs(a_results, b_ig, b_fg):
    """Concatenate per-core phase A outputs into full feature-major tensors."""
    def cat(name):
        return np.stack([
            np.concatenate([a_results[b * 4 + qt][name] for qt in range(4)], axis=1)
            for b in range(B)])
    q_t, k_t, v_t = cat('q_o'), cat('k_o'), cat('v_o')          # [B, INNER, S] bf16
    sxm_t, g2_t = cat('sxm_o'), cat('g2_o')
    gates = cat('gates_o').astype(np.float32)                   # [B, 16, S]
    i_pre = gates[:, :NH, :] + np.asarray(b_ig, np.float32)[None, :, None]
    f_pre = gates[:, NH:, :] + np.asarray(b_fg, np.float32)[None, :, None]
    return q_t, k_t, v_t, sxm_t, g2_t, i_pre, f_pre


def build_b_inmaps(q_t, k_t, v_t, g, wp):
    """Per-core phase B inputs.  Core c handles units (b, 2h) where
    b = c // 4, heads (2*(c%4), 2*(c%4)+1)."""
    in_maps = []
    for c in range(8):
        b, hp = c // 4, c % 4
        m = {'mask_i': wp['mask4']}
        for u in range(NUNIT):
            h = 2 * hp + u
            rs = slice(h * DH, (h + 1) * DH)
            kf = k_t[b, rs, :].astype(np.float32)               # [128, S] feat-major
            esc = g['esc'][b, h]                                # [S]
            cch_tok = np.repeat(g['cch'][b, h], LC)             # [S]
            m[f"qT{u}"] = np.ascontiguousarray(q_t[b, rs, :])
            m[f"kTc{u}"] = _bf(kf * (esc * cch_tok)[None, :])
            # token-major [p, c, d] / [p, c, e]
            kesc = (kf.T * esc[:, None]).reshape(NCH, LC, DH)
            m[f"kesc{u}"] = _bf(kesc.transpose(1, 0, 2).reshape(DH, NCH * DH))
            vone = np.empty((NCH, LC, NW), np.float32)
            vone[:, :, :DH] = v_t[b, rs, :].astype(np.float32).T.reshape(NCH, LC, DH)
            vone[:, :, DH] = 1.0
            m[f"vone{u}"] = _bf(vone.transpose(1, 0, 2).reshape(DH, NCH * NW))
            scal = np.empty((DH, 3, NCH), np.float32)
            scal[:, 0, :] = g['dec'][b, h][None, :]
            scal[:, 1, :] = g['e2'][b, h].T
            scal[:, 2, :] = g['e3'][b, h].T
            m[f"scal{u}"] = np.ascontiguousarray(scal.reshape(DH, 3 * NCH))
        in_maps.append(m)
    return in_maps


def build_c_inmaps(b_results, sxm_t, g2_t, wp):
    """Assemble h from phase B [p, c, e] layouts into feature-major h_t,
    fold the gating (hg = (h + skip*xm) * silu(x_og)) during the
    reshard, then per-core phase C inputs."""
    h_t = np.empty((B, INNER, S), dtype=np.float32)
    for c in range(8):
        b, hp = c // 4, c % 4
        for u in range(NUNIT):
            h = 2 * hp + u
            # [p, c, e] -> [e, c*LC + p]
            harr = np.asarray(b_results[c][f"h{u}"]).astype(np.float32)
            harr = harr.reshape(DH, NCH, DH)
            h_t[b, h * DH:(h + 1) * DH, :] = (
                harr.transpose(2, 1, 0).reshape(DH, S))
    hg_t = _bf((h_t + np.asarray(sxm_t, np.float32))
               * np.asarray(g2_t, np.float32))
    in_maps = []
    for c in range(8):
        b, qt = c // 4, c % 4
        ts = slice(qt * TOK, (qt + 1) * TOK)
        in_maps.append(dict(
            hg_i=np.ascontiguousarray(hg_t[b, :, ts]),
            wdT=wp['wdT']))
    return in_maps


def assemble_output(c_results):
    out = np.empty((B, S, D), np.float32)
    for c in range(8):
        b, qt = c // 4, c % 4
        out[b, qt * TOK:(qt + 1) * TOK, :] = \
            np.asarray(c_results[c]['out_o']).astype(np.float32).T
    return out


# ------------------------------------------------------------------ entry ----
from concourse.bass_utils import run_bass_kernel_spmd as _run_spmd

_CACHE = {}


def _programs():
    if 'a' not in _CACHE:
        _CACHE['a'] = build_phase_a()
        _CACHE['b'] = build_phase_b()
        _CACHE['c'] = build_phase_c()
    return _CACHE['a'], _CACHE['b'], _CACHE['c']


def kernel(x, W_up, Wq, Wk, Wv, W_ig, b_ig, W_fg, b_fg, conv_w, conv_b, skip,
           W_down):
    x = np.asarray(x, np.float32)
    nc_a, nc_b, nc_c = _programs()
    cores = list(range(8))
    wp = prep_weights(W_up, Wq, Wk, Wv, W_ig, W_fg, conv_w, conv_b, skip, W_down)
    a_maps = build_a_inmaps(x, wp)
    ra = _run_spmd(nc_a, a_maps, core_ids=cores).results
    q_t, k_t, v_t, sxm_t, g2_t, i_pre, f_pre = assemble_a_outputs(ra, b_ig, b_fg)
    g = host_gate_math(i_pre, f_pre)
    b_maps = build_b_inmaps(q_t, k_t, v_t, g, wp)
    rb = _run_spmd(nc_b, b_maps, core_ids=cores).results
    c_maps = build_c_inmaps(rb, sxm_t, g2_t, wp)
    rc = _run_spmd(nc_c, c_maps, core_ids=cores).results
    return assemble_output(rc)


# revision 46
# speedup vs baseline: 1.0139x; 1.0039x over previous
"""Trainium2 Bass kernel for nn_ChunkwiseMLSTM (B=2, S=8192, D=512, INNER=1024, NH=8).

kernel(**inputs) -> np.ndarray [2, 8192, 512] f32.

Three SPMD launches on 8 NeuronCores:
  A: token-sharded projections (up-proj, causal conv+SiLU split across
     DVE/Pool with 4x tensor_scalar taps, q/k/v, gate pre-acts, skip*xm,
     silu(x_og) fused into single ACT ops straight from PSUM).
  B: head-sharded chunkwise mLSTM (LC=128 chunks):
     - stage i: inter-chunk state updates U_c = kesc_c^T @ [v|1] (PE),
       strided PSUM->SBUF evacs (ACT/Pool), then per-unit e-split
       tensor_tensor_scan on DVE+Pool with broadcast-AP decay input.
     - stage ii: per 4-chunk blocks: S matmuls (PE), mask-mult, num
       matmuls, f32 den column extraction; den clamp math batched per
       16-chunk group; h scaling + group-batched DMA out (Pool queue).
  C: token-sharded output gating + down-projection, 512-token-sliced
     streaming; host pre-folds h+skip*xm during the B->C reshard.
Host between launches: gate-derived scan scalars (O(B*NH*S)), weight
pre-transposition, resharding.
"""
import os
os.environ.setdefault("JAX_COMPILATION_CACHE_DIR",
                      os.path.expanduser("~/.cache/jax_bass_cache"))
os.environ.setdefault("JAX_PERSISTENT_CACHE_MIN_ENTRY_SIZE_BYTES", "0")
os.environ.setdefault("JAX_PERSISTENT_CACHE_MIN_COMPILE_TIME_SECS", "0")

import sys
if '/opt/trn_rl_repo' not in sys.path:
    sys.path.insert(0, '/opt/trn_rl_repo')

import numpy as np
import ml_dtypes

import concourse.bass as bass
import concourse.tile as tile
from concourse import mybir, bacc

F32 = mybir.dt.float32
BF16 = mybir.dt.bfloat16
AF = mybir.ActivationFunctionType
OP = mybir.AluOpType

B, S, D = 2, 8192, 512
INNER, NH, KCONV = 1024, 8, 4
DH = 128
EPS = 1e-6
LC = 128           # chunk length used on device (math is chunk-size invariant)
NCH = S // LC      # 64
NW = 129           # [C | n] state width
QK_SCALE = DH ** -0.5
TOK = S // 4       # tokens per core in phases A/C = 2048
TH = TOK + (KCONV - 1)   # 2051 with conv halo
NUNIT = 2          # (b,h) units per core in phase B


def _bf(x):
    return np.ascontiguousarray(np.asarray(x, np.float32).astype(ml_dtypes.bfloat16))


def new_nc():
    return bacc.Bacc(None, target_bir_lowering=False, debug=False)


# ---------------------------------------------------------------- phase A ----
def build_phase_a():
    nc = new_nc()
    xt = nc.dram_tensor("xt", [D, TH], BF16, kind="ExternalInput")            # x[b].T slice (halo)
    wupT = nc.dram_tensor("wupT", [D, 2 * INNER], BF16, kind="ExternalInput")  # W_up.T
    wqkvT = nc.dram_tensor("wqkvT", [INNER, 3 * INNER], BF16, kind="ExternalInput")
    wgT = nc.dram_tensor("wgT", [INNER, 2 * NH], BF16, kind="ExternalInput")   # [Wig.T | Wfg.T]
    convw = nc.dram_tensor("convw", [DH, (INNER // DH) * KCONV], F32, kind="ExternalInput")
    convb = nc.dram_tensor("convb", [DH, INNER // DH], F32, kind="ExternalInput")
    skip_i = nc.dram_tensor("skip_i", [DH, INNER // DH], F32, kind="ExternalInput")

    q_o = nc.dram_tensor("q_o", [INNER, TOK], BF16, kind="ExternalOutput")
    k_o = nc.dram_tensor("k_o", [INNER, TOK], BF16, kind="ExternalOutput")
    v_o = nc.dram_tensor("v_o", [INNER, TOK], BF16, kind="ExternalOutput")
    sxm_o = nc.dram_tensor("sxm_o", [INNER, TOK], BF16, kind="ExternalOutput")   # skip * xm
    g2_o = nc.dram_tensor("g2_o", [INNER, TOK], BF16, kind="ExternalOutput")     # silu(x_og)
    gates_o = nc.dram_tensor("gates_o", [2 * NH, TOK], F32, kind="ExternalOutput")

    KT_UP = D // DH          # 4 k-tiles for up-proj
    FT = INNER // DH         # 8 feature tiles of the mlstm half
    KT_IN = INNER // DH      # 8 k-tiles over INNER
    MT_QKV = 3 * FT          # 24
    HALO = KCONV - 1         # 3

    with tile.TileContext(nc) as tc, \
         tc.tile_pool(name="const", bufs=1) as const, \
         tc.tile_pool(name="big", bufs=1) as big, \
         tc.tile_pool(name="ev", bufs=3) as ev, \
         tc.tile_pool(name="wpool", bufs=4) as wpool, \
         tc.tile_pool(name="gev", bufs=1) as gev, \
         tc.tile_pool(name="cv", bufs=3) as cv, \
         tc.tile_pool(name="ps", bufs=2, space="PSUM") as ps:
        if True:
            # --- load weights / x interleaved in PE consumption order:
            # wup m-slices and xt token-chunks arrive just ahead of the
            # up-projection chains that need them.
            wup_sb = big.tile([DH, KT_UP, 2 * INNER], BF16)
            xt_sb = big.tile([DH, KT_UP, TH], BF16)
            XCH = [(0, 515), (515, 512), (1027, 512), (1539, 512)]
            # PE m-tile order for the up-proj (mlstm front-loaded, og fills)
            UP_ORDER = [0, 1, 2, 8, 3, 9, 4, 10, 5, 11, 6, 12, 7, 13, 14, 15]

            def wup_slice(j):
                nc.sync.dma_start(
                    out=wup_sb[:, :, j * DH:(j + 1) * DH],
                    in_=wupT[:, j * DH:(j + 1) * DH].rearrange("(kt p) m -> p kt m", p=DH))
            for j in UP_ORDER[:4]:
                wup_slice(j)
            for c, (c0, cn) in enumerate(XCH):
                for kt in range(KT_UP):
                    nc.sync.dma_start(out=xt_sb[:, kt, c0:c0 + cn],
                                      in_=xt[kt * DH:(kt + 1) * DH, c0:c0 + cn])
            for j in UP_ORDER[4:]:
                wup_slice(j)
            convw_sb = const.tile([DH, FT, KCONV], F32)
            nc.sync.dma_start(out=convw_sb, in_=convw[:].rearrange("p (ft t) -> p ft t", ft=FT))
            convb_sb = const.tile([DH, FT], F32)
            nc.sync.dma_start(out=convb_sb, in_=convb[:])
            skip_sb = const.tile([DH, FT], F32)
            nc.sync.dma_start(out=skip_sb, in_=skip_i[:])
            wg_sb = const.tile([DH, KT_IN, 2 * NH], BF16)
            nc.sync.dma_start(out=wg_sb, in_=wgT[:].rearrange("(kt p) m -> p kt m", p=DH))

            xpre_sb = big.tile([DH, FT, TH], BF16)   # [p, ft, halo+tok]
            xm_sb = big.tile([DH, FT, TOK], BF16)
            xog_sb = big.tile([DH, FT, TOK], BF16)

            # --- halo chains (tokens 0..2 of xpre), one psum tile + one evac
            pt_h = ps.tile([DH, FT, HALO], F32, tag="mm")
            for m in range(FT):
                for kt in range(KT_UP):
                    nc.tensor.matmul(
                        pt_h[:, m, :],
                        wup_sb[:, kt, m * DH:(m + 1) * DH],
                        xt_sb[:, kt, 0:HALO],
                        start=(kt == 0), stop=(kt == KT_UP - 1))
            nc.scalar.copy(xpre_sb[:, :, 0:HALO], pt_h[:])

            rot = [0]
            def evac(dst, src, eng=None):
                # PSUM sources: only ACT/DVE may read PSUM (not GPSIMD)
                if eng is None:
                    eng = 'ad'[rot[0] % 2]
                    rot[0] += 1
                if eng == 'a':
                    nc.scalar.copy(dst, src)
                else:
                    nc.vector.tensor_copy(dst, src)

            # conv per ft: taps 0/2 as DVE tensor_scalar (4x mode), tap 1 on
            # Pool / tap 3 on DVE scalar_tensor_tensor, final add on DVE,
            # sigmoid on ACT, silu-mult on Pool.
            def emit_conv(ft):
                def xs(tau):
                    return xpre_sb[:, ft, tau:tau + TOK]
                # taps on DVE tensor_scalar (4x mode); conv bias folded into
                # tap 0; adds split DVE/Pool; silu-mult on Pool (SBUF only)
                p0 = cv.tile([DH, TOK], BF16, tag="ca")
                nc.vector.tensor_scalar(p0[:], xs(0), convw_sb[:, ft, 0:1],
                                        convb_sb[:, ft:ft + 1], OP.mult, OP.add)
                p1 = cv.tile([DH, TOK], BF16, tag="cb")
                nc.vector.tensor_scalar_mul(p1[:], xs(1), convw_sb[:, ft, 1:2])
                p2 = cv.tile([DH, TOK], BF16, tag="ca")
                nc.vector.tensor_scalar_mul(p2[:], xs(2), convw_sb[:, ft, 2:3])
                p3 = cv.tile([DH, TOK], BF16, tag="cb")
                nc.vector.tensor_scalar_mul(p3[:], xs(3), convw_sb[:, ft, 3:4])
                q0 = cv.tile([DH, TOK], BF16, tag="ca")
                nc.gpsimd.tensor_tensor(q0[:], p0[:], p1[:], OP.add)
                q1 = cv.tile([DH, TOK], BF16, tag="cb")
                nc.vector.tensor_tensor(q1[:], p2[:], p3[:], OP.add)
                y = cv.tile([DH, TOK], BF16, tag="y", bufs=2)
                nc.vector.tensor_tensor(y[:], q0[:], q1[:], OP.add)
                sg = cv.tile([DH, TOK], BF16, tag="sg", bufs=2)
                nc.scalar.activation(sg[:], y[:], AF.Sigmoid)
                nc.gpsimd.tensor_tensor(xm_sb[:, ft, :], y[:], sg[:], OP.mult)
                # sxm = skip * xm (DVE 4x)
                sxm_t = ev.tile([DH, TOK], BF16, tag="out")
                nc.vector.tensor_scalar_mul(sxm_t[:], xm_sb[:, ft, :],
                                            skip_sb[:, ft:ft + 1])
                nc.sync.dma_start(out=sxm_o[ft * DH:(ft + 1) * DH, :], in_=sxm_t[:])

            # --- up-projection in UP_ORDER (j<8: mlstm m-tile j, then its
            # conv; j>=8: og m-tile j-8, single-copy evac)
            for j in UP_ORDER:
                pt = ps.tile([DH, 4, 512], F32, tag="mm")
                for ns in range(4):
                    for kt in range(KT_UP):
                        nc.tensor.matmul(
                            pt[:, ns, :],
                            wup_sb[:, kt, j * DH:(j + 1) * DH],
                            xt_sb[:, kt, HALO + ns * 512: HALO + (ns + 1) * 512],
                            start=(kt == 0), stop=(kt == KT_UP - 1))
                if j < FT:
                    evac(xpre_sb[:, j, HALO:HALO + TOK],
                         pt[:].rearrange("p a b -> p (a b)"), eng='a')
                    emit_conv(j)
                else:
                    evac(xog_sb[:, j - FT, :], pt[:].rearrange("p a b -> p (a b)"),
                         eng='a' if (j % 2) else 'd')

            # --- g2 = silu(x_og) from SBUF (runs during gates/qkv PE work)
            for m in range(FT):
                sg2 = cv.tile([DH, TOK], BF16, tag="sg", bufs=2)
                nc.scalar.activation(sg2[:], xog_sb[:, m, :], AF.Sigmoid)
                g2_t = ev.tile([DH, TOK], BF16, tag="out")
                if m % 2 == 0:
                    nc.gpsimd.tensor_tensor(g2_t[:], xog_sb[:, m, :], sg2[:], OP.mult)
                else:
                    nc.vector.tensor_tensor(g2_t[:], xog_sb[:, m, :], sg2[:], OP.mult)
                nc.sync.dma_start(out=g2_o[m * DH:(m + 1) * DH, :], in_=g2_t[:])

            # --- gates: [16, TOK] f32
            ptg = ps.tile([2 * NH, 4, 512], F32, tag="mm")
            for ns in range(4):
                for kt in range(KT_IN):
                    nc.tensor.matmul(
                        ptg[:, ns, :], wg_sb[:, kt, :],
                        xm_sb[:, kt, ns * 512:(ns + 1) * 512],
                        start=(kt == 0), stop=(kt == KT_IN - 1))
            for hf in range(2):
                gv = gev.tile([2 * NH, TOK // 2], F32, tag="gv")
                nc.vector.tensor_copy(
                    gv[:], ptg[:, hf * 2:(hf + 1) * 2, :].rearrange("p a b -> p (a b)"))
                nc.sync.dma_start(
                    out=gates_o[:, hf * (TOK // 2):(hf + 1) * (TOK // 2)], in_=gv[:])

            # --- q/k/v projections (streamed weights)
            qkv_outs = [q_o, k_o, v_o]
            for m in range(MT_QKV):
                # weights streamed on the ACT queue so they never wait
                # behind the big output DMAs on SP
                w_sb = wpool.tile([DH, KT_IN, DH], BF16, tag="w")
                nc.sync.dma_start(
                    out=w_sb,
                    in_=wqkvT[:, m * DH:(m + 1) * DH].rearrange("(kt p) m -> p kt m", p=DH))
                out_t = qkv_outs[m // FT]
                mf = m % FT
                pt = ps.tile([DH, 4, 512], F32, tag="mm")
                for ns in range(4):
                    for kt in range(KT_IN):
                        nc.tensor.matmul(
                            pt[:, ns, :], w_sb[:, kt, :],
                            xm_sb[:, kt, ns * 512:(ns + 1) * 512],
                            start=(kt == 0), stop=(kt == KT_IN - 1))
                ev_t = ev.tile([DH, TOK], BF16, tag="ev")
                evac(ev_t[:], pt[:].rearrange("p a b -> p (a b)"))
                nc.sync.dma_start(out=out_t[mf * DH:(mf + 1) * DH, :], in_=ev_t[:])
    nc.compile()
    return nc


# ---------------------------------------------------------------- phase B ----
def build_phase_b():
    nc = new_nc()
    ins = {}
    outs = {}
    for u in range(NUNIT):
        # feat-major q and (esc*cch)-scaled k
        ins[f"qT{u}"] = nc.dram_tensor(f"qT{u}", [DH, S], BF16, kind="ExternalInput")
        ins[f"kTc{u}"] = nc.dram_tensor(f"kTc{u}", [DH, S], BF16, kind="ExternalInput")
        # token-major (p = token-in-chunk): [p, c, d] esc-scaled k, [p, c, e] = [v | 1]
        ins[f"kesc{u}"] = nc.dram_tensor(f"kesc{u}", [DH, NCH * DH], BF16, kind="ExternalInput")
        ins[f"vone{u}"] = nc.dram_tensor(f"vone{u}", [DH, NCH * NW], BF16, kind="ExternalInput")
        # packed per-unit scalars: [p, {dec(col0=0), e2, e3}, NCH] f32
        ins[f"scal{u}"] = nc.dram_tensor(f"scal{u}", [DH, 3 * NCH], F32, kind="ExternalInput")
        # h out in [p, c, e] layout
        outs[f"h{u}"] = nc.dram_tensor(f"h{u}", [DH, NCH * DH], BF16, kind="ExternalOutput")
    mask_i = nc.dram_tensor("mask_i", [DH, 4 * DH], BF16, kind="ExternalInput")
    BDBG = bool(os.environ.get("BDBG"))
    if BDBG:
        for nm in ("dbgU0", "dbgCs0", "dbgdecf0", "dbgsp0", "dbgdraw0"):
            sz = NW * NCH if nm not in ("dbgsp0", "dbgdraw0") else 4 * DH * 4
            outs[nm] = nc.dram_tensor(nm, [DH, sz], BF16, kind="ExternalOutput")

    SB = 4          # chunks per block
    GRP = 2         # blocks per den/h-DMA group
    NE = NW * NCH   # 8256 elements per big ring buffer
    ESPLIT = 65     # scan e-split: DVE does e<65, Pool e>=65
    with tile.TileContext(nc) as tc, \
         tc.tile_pool(name="small", bufs=1) as small, \
         tc.tile_pool(name="sh", bufs=10) as sh, \
         tc.tile_pool(name="spb", bufs=6) as spb, \
         tc.tile_pool(name="hrb", bufs=8) as hrb, \
         tc.tile_pool(name="hgo", bufs=6) as hgo, \
         tc.tile_pool(name="den", bufs=2) as den, \
         tc.tile_pool(name="ps1", bufs=2, space="PSUM") as ps1, \
         tc.tile_pool(name="psn", bufs=3, space="PSUM") as psn:
        if True:
            def ring(name):
                return sh.tile([DH, NE], BF16, tag="sh", name=name)

            mask_sb = small.tile([DH, SB, DH], BF16, name="mask")
            nc.sync.dma_start(
                out=mask_sb, in_=mask_i[:].rearrange("p (b l) -> p b l", b=SB))
            T = {}
            for u in range(NUNIT):
                T[u] = dict(scal=small.tile([DH, 3, NCH], F32, name=f"scal{u}"))
                nc.sync.dma_start(
                    out=T[u]['scal'],
                    in_=ins[f"scal{u}"][:].rearrange("p (k c) -> p k c", k=3))
            # ring allocation order (11 bufs): kesc1 (12th) wraps onto kesc0,
            # which dies after stage_i(0).
            T[0]['kesc'] = ring("kesc0")
            T[0]['U'] = ring("U0")
            T[0]['vone'] = ring("vone0")
            T[0]['Cs'] = ring("Cs0")
            T[0]['qT'] = ring("qT0")
            T[0]['kTc'] = ring("kTc0")
            T[1]['vone'] = ring("vone1")
            T[1]['U'] = ring("U1")
            T[1]['Cs'] = ring("Cs1")
            T[1]['qT'] = ring("qT1")
            # wraps (11th/12th allocs -> slots 1/2): kesc1 -> kesc0 (dead
            # after stage_i(0)); kTc1 -> U0 (dead after scan(0))
            T[1]['kesc'] = ring("kesc1")
            T[1]['kTc'] = ring("kTc1")
            # decay tile shared by both units (rebuilt between scans)
            decf_sh = small.tile([DH, NE], BF16, name="decf_sh")
            T[0]['decf'] = decf_sh
            T[1]['decf'] = decf_sh

            HALF = NCH // 2
            def issue_unit_dmas(u):
                # all inputs on SP: u1's ring-slot waits only delay u1
                # issues, which are transfer-bound anyway
                eng = nc.sync
                for half in range(2):
                    ks = slice(half * HALF * DH, (half + 1) * HALF * DH)
                    vs = slice(half * HALF * NW, (half + 1) * HALF * NW)
                    eng.dma_start(out=T[u]['kesc'][:, ks],
                                  in_=ins[f"kesc{u}"][:, ks])
                    eng.dma_start(out=T[u]['vone'][:, vs],
                                  in_=ins[f"vone{u}"][:, vs])
                order = ('qT', 'kTc') if u == 0 else ('kTc', 'qT')
                for half in range(2):
                    ts = slice(half * (S // 2), (half + 1) * (S // 2))
                    for nm in order:
                        eng.dma_start(out=T[u][nm][:, ts],
                                      in_=ins[f"{nm}{u}"][:, ts])

            def kescv(u):
                return T[u]['kesc'][:, :NCH * DH].rearrange("p (c d) -> p c d", c=NCH)

            def vonev(u):
                return T[u]['vone'][:].rearrange("p (c e) -> p c e", c=NCH)

            def Uv(u):
                return T[u]['U'][:].rearrange("p (e c) -> p e c", e=NW)

            def Csv(u):
                return T[u]['Cs'][:].rearrange("p (e c) -> p e c", e=NW)

            def stage_i(u, half):
                for cb in range(half * (HALF // SB), (half + 1) * (HALF // SB)):
                    up = psn.tile([DH, SB, 256], F32, tag="nps")
                    for i in range(SB):
                        c = cb * SB + i
                        nc.tensor.matmul(
                            up[:, i, :NW], kescv(u)[:, c, :], vonev(u)[:, c, :],
                            start=True, stop=True)
                    # PSUM -> SBUF bf16, strided into [p, e, c] layout
                    dst = Uv(u)[:, :, cb * SB:(cb + 1) * SB].rearrange("p e c -> p c e")
                    nc.scalar.copy(dst, up[:, :, :NW])

            def build_decf(u):
                # decf[p, (e, c)] = dec[p, c]; col c=0 is 0 => resets the
                # carried state at each e-boundary during the scan.
                t = T[u]
                dec_bc = t['scal'][:, 0, :]
                c0 = ESPLIT * NCH
                nc.vector.tensor_copy(
                    t['decf'][:, :c0].rearrange("p (e c) -> p e c", e=ESPLIT),
                    dec_bc.unsqueeze(1).broadcast_to([DH, ESPLIT, NCH]))
                nc.gpsimd.tensor_copy(
                    t['decf'][:, c0:].rearrange("p (e c) -> p e c", e=NW - ESPLIT),
                    dec_bc.unsqueeze(1).broadcast_to([DH, NW - ESPLIT, NCH]))

            def scan(u):
                # scans are DVE-only on HW (TensorScalarPtr engine check)
                t = T[u]
                nc.vector.tensor_tensor_scan(
                    t['Cs'][:], t['decf'][:], t['U'][:],
                    0.0, OP.mult, OP.add)

            def stage_ii_group(u, g):
                t = T[u]
                c0 = g * GRP * SB                 # first chunk of group
                draw = den.tile([DH, GRP * SB], F32, tag="draw")
                hrs = []
                for blk in range(GRP):
                    cb = g * GRP + blk
                    sps = ps1.tile([DH, SB, DH], F32, tag="sps")
                    for i in range(SB):
                        c = cb * SB + i
                        csl = slice(c * LC, (c + 1) * LC)
                        nc.tensor.matmul(
                            sps[:, i, :], t['kTc'][:, csl], t['qT'][:, csl],
                            start=True, stop=True)
                    # Sp = S * mask (esc/cch scaling baked into kTc)
                    sp = spb.tile([DH, SB, DH], BF16, tag="sp")
                    nc.vector.tensor_tensor(sp[:], sps[:], mask_sb[:], OP.mult)
                    # num matmuls
                    if BDBG and u == 0 and g == 0 and blk == 0:
                        nc.sync.dma_start(out=outs["dbgsp0"][:, :SB * DH].rearrange("p (a b) -> p a b", a=SB), in_=sp[:])
                    nps = psn.tile([DH, SB, 256], F32, tag="nps")
                    for i in range(SB):
                        c = cb * SB + i
                        csl = slice(c * LC, (c + 1) * LC)
                        if c > 0:
                            nc.tensor.matmul(
                                nps[:, i, :NW], t['qT'][:, csl], Csv(u)[:, :, c - 1],
                                start=True, stop=False)
                        nc.tensor.matmul(
                            nps[:, i, :NW], sp[:, i, :], vonev(u)[:, c, :],
                            start=(c == 0), stop=True)
                    # raw den column (f32) + unscaled h (bf16) out of PSUM
                    nc.scalar.copy(
                        draw[:, blk * SB:(blk + 1) * SB], nps[:, :, DH])
                    hr = hrb.tile([DH, SB, DH], BF16, tag="hr")
                    if blk % 2 == 0:
                        nc.scalar.copy(hr[:], nps[:, :, :DH])
                    else:
                        nc.vector.tensor_copy(hr[:], nps[:, :, :DH])
                    hrs.append(hr)
                if BDBG and u == 0 and g == 0:
                    nc.gpsimd.dma_start(out=outs["dbgdraw0"][:, :GRP * SB],
                                      in_=draw[:])
                # batched den for the 16-chunk group: den = max(|raw|, e2) + e3
                gsl = slice(c0, c0 + GRP * SB)
                dabs = den.tile([DH, GRP * SB], F32, tag="dabs")
                nc.scalar.activation(dabs[:], draw[:], AF.Abs)
                dmx = den.tile([DH, GRP * SB], F32, tag="dmx")
                nc.vector.tensor_tensor(dmx[:], dabs[:], t['scal'][:, 1, gsl], OP.max)
                dpl = den.tile([DH, GRP * SB], F32, tag="dpl")
                nc.vector.tensor_tensor(dpl[:], dmx[:], t['scal'][:, 2, gsl], OP.add)
                rden = den.tile([DH, GRP * SB], F32, tag="rden")
                nc.vector.reciprocal(rden[:], dpl[:])
                # h = hr * rden (broadcast over d), batched out-DMA per group
                hg = hgo.tile([DH, GRP * SB, DH], BF16, tag="hg")
                for blk in range(GRP):
                    bsl = slice(blk * SB, (blk + 1) * SB)
                    rb = rden[:, bsl].unsqueeze(2).broadcast_to([DH, SB, DH])
                    nc.gpsimd.tensor_tensor(hg[:, bsl, :], hrs[blk][:], rb, OP.mult)
                nc.sync.dma_start(
                    out=outs[f"h{u}"][:, c0 * DH:(c0 + GRP * SB) * DH], in_=hg[:])

            issue_unit_dmas(0)
            issue_unit_dmas(1)
            with tc.high_priority():
                build_decf(0)
                stage_i(0, 0)
                stage_i(0, 1)
                scan(0)
            if BDBG:
                for nm, t_ in (("dbgU0", T[0]['U']), ("dbgCs0", T[0]['Cs']),
                               ("dbgdecf0", T[0]['decf'])):
                    nc.sync.dma_start(out=outs[nm][:], in_=t_[:])
            NG = NCH // (GRP * SB)
            u0_pre = list(range(NG // 2))           # first half of u0 groups
            for g in u0_pre:
                stage_ii_group(0, g)
            stage_i(1, 0)
            stage_ii_group(0, NG // 2)
            stage_i(1, 1)
            build_decf(1)
            scan(1)
            for g in range(NG // 2 + 1, NG):
                stage_ii_group(0, g)
            for g in range(NG):
                stage_ii_group(1, g)
    nc.compile()
    return nc


# ---------------------------------------------------------------- phase C ----
def build_phase_c():
    nc = new_nc()
    hg_i = nc.dram_tensor("hg_i", [INNER, TOK], BF16, kind="ExternalInput")  # (h+skip*xm)*silu(x_og)
    wdT = nc.dram_tensor("wdT", [INNER, D], BF16, kind="ExternalInput")
    out_o = nc.dram_tensor("out_o", [D, TOK], BF16, kind="ExternalOutput")

    FT = INNER // DH   # 8
    MT = D // DH       # 4
    NS = TOK // 512    # 4 token slices
    with tile.TileContext(nc) as tc, \
         tc.tile_pool(name="big", bufs=1) as big, \
         tc.tile_pool(name="ev", bufs=4) as ev, \
         tc.tile_pool(name="ps", bufs=4, space="PSUM") as ps:
        if True:
            wd_sb = big.tile([DH, FT, D], BF16)
            hg_sb = big.tile([DH, FT, TOK], BF16)
            # weight m-tile 0 first so the first chain starts early
            nc.sync.dma_start(out=wd_sb[:, :, :DH],
                              in_=wdT[:, :DH].rearrange("(ft p) m -> p ft m", p=DH))
            for ns in range(NS):
                tsl = slice(ns * 512, (ns + 1) * 512)
                nc.sync.dma_start(
                    out=hg_sb[:, :, tsl],
                    in_=hg_i[:, tsl].rearrange("(ft p) t -> p ft t", p=DH))
                if ns == 0:
                    nc.sync.dma_start(
                        out=wd_sb[:, :, DH:],
                        in_=wdT[:, DH:].rearrange("(ft p) m -> p ft m", p=DH))
            for ns in range(NS):
                tsl = slice(ns * 512, (ns + 1) * 512)
                for m in range(MT):
                    pt = ps.tile([DH, 512], F32)
                    for kt in range(FT):
                        nc.tensor.matmul(
                            pt[:], wd_sb[:, kt, m * DH:(m + 1) * DH],
                            hg_sb[:, kt, tsl],
                            start=(kt == 0), stop=(kt == FT - 1))
                    ot = ev.tile([DH, 512], BF16, tag="ot")
                    if m % 2 == 0:
                        nc.scalar.copy(ot[:], pt[:])
                    else:
                        nc.vector.tensor_copy(ot[:], pt[:])
                    nc.sync.dma_start(
                        out=out_o[m * DH:(m + 1) * DH, tsl], in_=ot[:])
    nc.compile()
    return nc


# ------------------------------------------------------------- host glue ----
def host_gate_math(i_pre, f_pre):
    """i_pre, f_pre: [B, NH, S] f32.  Returns dict of f32 arrays.

    Exports (per b, h):
      esc   [S]        exp(a_j - m_new(chunk))            per-token k scale
      cch   [NCH]      exp(m_new - ms - scaG)             per-chunk Sp scale
      dec   [NCH]      exp(scaG + ms - m_new), col0 = 0   scan multiplier
      e2,e3 [NCH, LC]  den clamp terms (qk_scale folded)
    """
    i_pre = i_pre.astype(np.float64)
    f_pre = f_pre.astype(np.float64)
    vecI = np.log(1.0 / (1.0 + np.exp(-i_pre)) + EPS)
    vecF = np.log(1.0 / (1.0 + np.exp(-f_pre)) + EPS)
    Ic = vecI.reshape(B, NH, NCH, LC)
    Fc = vecF.reshape(B, NH, NCH, LC)
    vecB = np.cumsum(Fc, axis=-1)
    scaG = vecB[..., -1]
    vecA = scaG[..., None] - vecB + Ic

    ms = np.zeros((B, NH, NCH))
    dec = np.zeros((B, NH, NCH))
    m_new_arr = np.zeros((B, NH, NCH))
    m = np.zeros((B, NH))
    for c in range(NCH):
        amax = vecA[:, :, c, :].max(-1)
        m_new = np.maximum(scaG[:, :, c] + m, amax)
        ms[:, :, c] = m
        dec[:, :, c] = np.exp(scaG[:, :, c] + m - m_new)
        m_new_arr[:, :, c] = m_new
        m = m_new
    escale = np.exp(vecA - m_new_arr[..., None])          # [B,NH,NCH,LC]
    cch = np.exp(m_new_arr - ms - scaG)                   # [B,NH,NCH]

    mask = np.tril(np.ones((LC, LC), bool))
    logD = vecB[..., :, None] - vecB[..., None, :] + Ic[..., None, :]
    logD = np.where(mask, logD, -np.inf)
    m_intra = logD.max(-1)
    m_comb = np.maximum(vecB + ms[..., None], m_intra)
    e2 = np.exp(-vecB - ms[..., None]) / QK_SCALE
    e3 = EPS * np.exp(m_comb - vecB - ms[..., None]) / QK_SCALE
    dec0 = dec.copy()
    dec0[:, :, 0] = 0.0
    return dict(
        esc=escale.reshape(B, NH, S).astype(np.float32),
        cch=cch.astype(np.float32),
        dec=dec0.astype(np.float32),
        e2=e2.astype(np.float32), e3=e3.astype(np.float32))


def prep_weights(W_up, Wq, Wk, Wv, W_ig, W_fg, conv_w, conv_b, skip, W_down):
    """Host-side weight packing (same for all cores)."""
    FT = INNER // DH
    wupT = _bf(W_up.T)                                         # [512, 2048]
    wqkvT = _bf(np.concatenate([Wq.T, Wk.T, Wv.T], axis=1))    # [1024, 3072]
    wgT = _bf(np.concatenate([W_ig.T, W_fg.T], axis=1))        # [1024, 16]
    convw = np.ascontiguousarray(
        conv_w.reshape(FT, DH, KCONV).transpose(1, 0, 2).reshape(DH, FT * KCONV)
    ).astype(np.float32)
    convb = np.ascontiguousarray(conv_b.reshape(FT, DH).T).astype(np.float32)
    skip_p = np.ascontiguousarray(skip.reshape(FT, DH).T).astype(np.float32)
    wdT = _bf(W_down.T)                                        # [1024, 512]
    mask4 = _bf(np.tile(np.tril(np.ones((DH, DH), np.float32)), (1, 4)))
    return dict(wupT=wupT, wqkvT=wqkvT, wgT=wgT, convw=convw, convb=convb,
                skip_p=skip_p, wdT=wdT, mask4=mask4)


def build_a_inmaps(x, wp):
    """Per-core phase A input maps.  Core c = (b=c//4, quarter=c%4)."""
    in_maps = []
    for c in range(8):
        b, qt = c // 4, c % 4
        s0 = qt * TOK
        xs = x[b, :, :].T                                       # [512, S] view
        if s0 == 0:
            xt = np.concatenate([np.zeros((D, KCONV - 1), np.float32),
                                 xs[:, :TOK]], axis=1)
        else:
            xt = xs[:, s0 - (KCONV - 1): s0 + TOK]
        in_maps.append(dict(
            xt=_bf(xt), wupT=wp['wupT'], wqkvT=wp['wqkvT'], wgT=wp['wgT'],
            convw=wp['convw'], convb=wp['convb'], skip_i=wp['skip_p']))
    return in_maps


def assemble_a_outputs(a_results, b_ig, b_fg):
    """Concatenate per-core phase A outputs into full feature-major tensors."""
    def cat(name):
        return np.stack([
            np.concatenate([a_results[b * 4 + qt][name] for qt in range(4)], axis=1)
            for b in range(B)])
    q_t, k_t, v_t = cat('q_o'), cat('k_o'), cat('v_o')          # [B, INNER, S] bf16
    sxm_t, g2_t = cat('sxm_o'), cat('g2_o')
    gates = cat('gates_o').astype(np.float32)                   # [B, 16, S]
    i_pre = gates[:, :NH, :] + np.asarray(b_ig, np.float32)[None, :, None]
    f_pre = gates[:, NH:, :] + np.asarray(b_fg, np.float32)[None, :, None]
    return q_t, k_t, v_t, sxm_t, g2_t, i_pre, f_pre


def build_b_inmaps(q_t, k_t, v_t, g, wp):
    """Per-core phase B inputs.  Core c handles units (b, 2h) where
    b = c // 4, heads (2*(c%4), 2*(c%4)+1)."""
    in_maps = []
    for c in range(8):
        b, hp = c // 4, c % 4
        m = {'mask_i': wp['mask4']}
        for u in range(NUNIT):
            h = 2 * hp + u
            rs = slice(h * DH, (h + 1) * DH)
            kf = k_t[b, rs, :].astype(np.float32)               # [128, S] feat-major
            esc = g['esc'][b, h]                                # [S]
            cch_tok = np.repeat(g['cch'][b, h], LC)             # [S]
            m[f"qT{u}"] = np.ascontiguousarray(q_t[b, rs, :])
            m[f"kTc{u}"] = _bf(kf * (esc * cch_tok)[None, :])
            # token-major [p, c, d] / [p, c, e]
            kesc = (kf.T * esc[:, None]).reshape(NCH, LC, DH)
            m[f"kesc{u}"] = _bf(kesc.transpose(1, 0, 2).reshape(DH, NCH * DH))
            vone = np.empty((NCH, LC, NW), np.float32)
            vone[:, :, :DH] = v_t[b, rs, :].astype(np.float32).T.reshape(NCH, LC, DH)
            vone[:, :, DH] = 1.0
            m[f"vone{u}"] = _bf(vone.transpose(1, 0, 2).reshape(DH, NCH * NW))
            scal = np.empty((DH, 3, NCH), np.float32)
            scal[:, 0, :] = g['dec'][b, h][None, :]
            scal[:, 1, :] = g['e2'][b, h].T
            scal[:, 2, :] = g['e3'][b, h].T
            m[f"scal{u}"] = np.ascontiguousarray(scal.reshape(DH, 3 * NCH))
        in_maps.append(m)
    return in_maps


def build_c_inmaps(b_results, sxm_t, g2_t, wp):
    """Assemble h from phase B [p, c, e] layouts into feature-major h_t,
    fold the gating (hg = (h + skip*xm) * silu(x_og)) during the
    reshard, then per-core phase C inputs."""
    h_t = np.empty((B, INNER, S), dtype=np.float32)
    for c in range(8):
        b, hp = c // 4, c % 4
        for u in range(NUNIT):
            h = 2 * hp + u
            # [p, c, e] -> [e, c*LC + p]
            harr = np.asarray(b_results[c][f"h{u}"]).astype(np.float32)
            harr = harr.reshape(DH, NCH, DH)
            h_t[b, h * DH:(h + 1) * DH, :] = (
                harr.transpose(2, 1, 0).reshape(DH, S))
    hg_t = _bf((h_t + np.asarray(sxm_t, np.float32))
               * np.asarray(g2_t, np.float32))
    in_maps = []
    for c in range(8):
        b, qt = c // 4, c % 4
        ts = slice(qt * TOK, (qt + 1) * TOK)
        in_maps.append(dict(
            hg_i=np.ascontiguousarray(hg_t[b, :, ts]),
            wdT=wp['wdT']))
    return in_maps


def assemble_output(c_results):
    out = np.empty((B, S, D), np.float32)
    for c in range(8):
        b, qt = c // 4, c % 4
        out[b, qt * TOK:(qt + 1) * TOK, :] = \
            np.asarray(c_results[c]['out_o']).astype(np.float32).T
    return out


# ------------------------------------------------------------------ entry ----
from concourse.bass_utils import run_bass_kernel_spmd as _run_spmd

_CACHE = {}


def _programs():
    if 'a' not in _CACHE:
        _CACHE['a'] = build_phase_a()
        _CACHE['b'] = build_phase_b()
        _CACHE['c'] = build_phase_c()
    return _CACHE['a'], _CACHE['b'], _CACHE['c']


def kernel(x, W_up, Wq, Wk, Wv, W_ig, b_ig, W_fg, b_fg, conv_w, conv_b, skip,
           W_down):
    x = np.asarray(x, np.float32)
    nc_a, nc_b, nc_c = _programs()
    cores = list(range(8))
    wp = prep_weights(W_up, Wq, Wk, Wv, W_ig, W_fg, conv_w, conv_b, skip, W_down)
    a_maps = build_a_inmaps(x, wp)
    ra = _run_spmd(nc_a, a_maps, core_ids=cores).results
    q_t, k_t, v_t, sxm_t, g2_t, i_pre, f_pre = assemble_a_outputs(ra, b_ig, b_fg)
    g = host_gate_math(i_pre, f_pre)
    b_maps = build_b_inmaps(q_t, k_t, v_t, g, wp)
    rb = _run_spmd(nc_b, b_maps, core_ids=cores).results
    c_maps = build_c_inmaps(rb, sxm_t, g2_t, wp)
    rc = _run_spmd(nc_c, c_maps, core_ids=cores).results
    return assemble_output(rc)
